# revision 38
# baseline (speedup 1.0000x reference)
"""Trainium2 Bass kernel for nn_Attention_28802050687686.

GQA sliding-window attention, T=4096, D=2048, 8 Q heads / 4 KV heads,
head_dim 256, window 1024, tanh soft-cap 50, RMSNorm+RoPE on Q/K, RMSNorm on V.

Sequence-parallel over 8 cores (512 query rows each). K/V computed locally,
AllGathered (window halo read back via partition-id offsets). Projections and
the output matmul run in fp8 DoubleRow (2x); the first 128 global query rows
are recomputed in bf16 by every core from identical inputs and blended in on
core 0 only (fp8 noise does not average down over tiny attention windows).
RMSNorm scale vectors are folded host-side into rope tables (q/k) and o_w (v).
"""
import sys

sys.path.insert(0, "/opt/trn_rl_repo")

import numpy as np
import ml_dtypes

import concourse.bass as bass
import concourse.tile as tile
from concourse import bacc, mybir
from concourse.bass_utils import run_bass_kernel_spmd

F32 = mybir.dt.float32
BF16 = mybir.dt.bfloat16
F8 = mybir.dt.float8e4
AF = mybir.ActivationFunctionType
OP = mybir.AluOpType
DR = mybir.MatmulPerfMode.DoubleRow

W_SCALE = 64.0     # fp8 range scale folded into q/k/v/o weights
V_SCALE = 32.0     # scale folded into normalized V (and out of final output)

# problem constants
T, D, NH, KV, H, HH = 4096, 2048, 8, 4, 256, 128
N_CORES = 8
TC = 512          # queries / own kv rows per core
SW = 1536         # kv window per core
NST = SW // 128   # 12 s-tiles in window
NOT = TC // 128   # 4 own s-tiles
NDT = D // 128    # 16 d-tiles
NTT = TC // 128   # 4 t-tiles
WINDOW = 1024
SOFT_CAP = 50.0
EPS = 1e-6
ROPE_BASE = 10000.0

KCOLS = NH * TC            # K pack: 8 planes (4 kv x 2 halves) x 512 own rows
VCOLS = NOT * KV * 256     # V pack
# valid query-column range per window s-tile j (sliding window trapezoid)
ST_LO = [max(0, 128 * (j - 8)) for j in range(NST)]
ST_HI = [min(TC, 128 * (j + 1)) for j in range(NST)]
ST_ORDER = [8, 9, 10, 11, 0, 1, 2, 3, 4, 5, 6, 7]  # own-block tiles first


def build_program():
    nc = bacc.Bacc("TRN2", target_bir_lowering=False, debug=False)

    xq = nc.dram_tensor("xq", [D, TC], F8, kind="ExternalInput").ap()
    qw = nc.dram_tensor("qw", [D, NH * H], F8, kind="ExternalInput").ap()
    kwk = nc.dram_tensor("kwk", [D, KV * H], F8, kind="ExternalInput").ap()
    kwv = nc.dram_tensor("kwv", [D, KV * H], F8, kind="ExternalInput").ap()
    ow = nc.dram_tensor("ow", [NH * H, D], F8, kind="ExternalInput").ap()
    ow16 = nc.dram_tensor("ow16", [NH * H, D], BF16, kind="ExternalInput").ap()
    qtab = nc.dram_tensor("qtab", [8, HH, TC], BF16, kind="ExternalInput").ap()
    maskT = nc.dram_tensor("maskT", [NST, 128, TC], F32,
                           kind="ExternalInput").ap()
    x16 = nc.dram_tensor("x16", [D, 128], BF16, kind="ExternalInput").ap()
    wfixq = nc.dram_tensor("wfixq", [D, 2048], BF16, kind="ExternalInput").ap()
    wfixkv = nc.dram_tensor("wfixkv", [D, 2048], BF16,
                            kind="ExternalInput").ap()
    ftq = nc.dram_tensor("ftq", [128, 4 * 8 * 128], BF16,
                         kind="ExternalInput").ap()
    ftk = nc.dram_tensor("ftk", [128, 4 * 4 * 128], BF16,
                         kind="ExternalInput").ap()
    maskf = nc.dram_tensor("maskf", [128, 128], BF16, kind="ExternalInput").ap()
    fixmask = nc.dram_tensor("fixmask", [128, 1], F32, kind="ExternalInput").ap()
    ident = nc.dram_tensor("ident", [128, 128], BF16, kind="ExternalInput").ap()
    out = nc.dram_tensor("out", [TC, D], F32, kind="ExternalOutput").ap()

    klocal = nc.dram_tensor("klocal", [128, KCOLS], BF16).ap()
    kgath = nc.dram_tensor("kgath", [N_CORES * 128, KCOLS], BF16,
                           addr_space="Shared").ap()
    vlocal = nc.dram_tensor("vlocal", [128, VCOLS], BF16).ap()
    vgath = nc.dram_tensor("vgath", [N_CORES * 128, VCOLS], BF16,
                           addr_space="Shared").ap()

    NORM_BIAS = EPS * W_SCALE * W_SCALE          # sqrt(rps/H + this)
    with tile.TileContext(nc) as tc:
        with tc.tile_pool(name="persist", bufs=1) as persist:
            kT_sb = persist.tile([128, KV * 2, SW], BF16)     # 24 KB/p
            V_sb = persist.tile([128, NST, KV, 256], BF16)    # 24 KB/p
            qT_g = [persist.tile([128, 4, TC], BF16, name=f"qT{g}")
                    for g in range(KV)]
            encT_sb = persist.tile([128, NH * 2, TC], F8)     # 8 KB/p
            enc0_16 = persist.tile([128, NH * 2, 128], BF16)  # 4 KB/p
            encF_16 = persist.tile([128, NH * 2, 128], BF16)  # 4 KB/p

            # -------- input loads, priority order (sync queue) --------
            xq_ch = []
            for ch in range(4):
                xc = persist.tile([128, NDT // 4, TC], F8, name=f"xq{ch}")
                nc.sync.dma_start(
                    xc[:], xq[ch * (D // 4):(ch + 1) * (D // 4), :].rearrange(
                        "(dt p) s -> p dt s", p=128))
                xq_ch.append(xc)
            qtab_sb = persist.tile([128, 8, TC], BF16)        # 8 KB/p
            nc.sync.dma_start(qtab_sb[:], qtab.rearrange("f p t -> p f t"))
            x16_sb = persist.tile([128, NDT, 128], BF16)      # 4 KB/p
            nc.sync.dma_start(
                x16_sb[:], x16[:].rearrange("(dt p) s -> p dt s", p=128))
            ftq_sb = persist.tile([128, 4, 8, 128], BF16)     # 8 KB/p
            nc.sync.dma_start(
                ftq_sb[:], ftq[:].rearrange("p (f r h) -> p f r h", f=4, r=8))
            ftk_sb = persist.tile([128, 4, 4, 128], BF16)     # 4 KB/p
            nc.sync.dma_start(
                ftk_sb[:], ftk[:].rearrange("p (f r h) -> p f r h", f=4, r=4))
            maskf_sb = persist.tile([128, 128], BF16)
            nc.sync.dma_start(maskf_sb[:], maskf[:])
            fixm_sb = persist.tile([128, 1], F32)
            nc.sync.dma_start(fixm_sb[:], fixmask[:])
            ident_sb = persist.tile([128, 128], BF16)
            nc.sync.dma_start(ident_sb[:], ident[:])

            ones_f = persist.tile([1, 128], BF16)
            nc.vector.memset(ones_f[:], 1.0)
            ones_b = persist.tile([128, 1], BF16)
            nc.vector.memset(ones_b[:], 1.0)
            nbias1 = persist.tile([1, 1], F32)
            nc.vector.memset(nbias1[:], EPS * W_SCALE * W_SCALE)
            epsv = persist.tile([128, 1], F32)
            nc.vector.memset(epsv[:], EPS / (V_SCALE * V_SCALE))
            epsvm = persist.tile([128, 1], F32)
            nc.vector.memset(epsvm[:], EPS * (W_SCALE / V_SCALE) ** 2)
            nbiasq = persist.tile([1, 1], F32)
            nc.vector.memset(nbiasq[:], EPS * W_SCALE * W_SCALE * H)
            epsq128 = persist.tile([128, 1], F32)
            nc.vector.memset(epsq128[:], EPS * H)
            eps1 = persist.tile([128, 1], F32)
            nc.vector.memset(eps1[:], EPS)

            def xq_pair(j):
                ch, o = divmod(2 * j, 4)
                return xq_ch[ch][:, o:o + 2, :]

            def rope_norm_fold(ps_pair, tb, dst0, dst1):
                """RMSNorm + rope on raw pair [128,2,TC] f32 PSUM; tb = index
                of the 4-table block in qtab_sb (0 for Q, 4 for K)."""
                braw = work.tile([128, 2, TC], BF16, tag="braw", name="braw")
                nc.vector.tensor_copy(braw[:], ps_pair[:])
                sq = work.tile([128, 2, TC], BF16, tag="wsq", name="sq")
                nc.vector.tensor_tensor(sq[:], braw[:], braw[:], OP.mult)
                rps = ps12.tile([1, TC], F32, tag="rowps", name="rps")
                nc.tensor.matmul(rps[:], ones_b[:], sq[:, 0, :],
                                 start=True, stop=False)
                nc.tensor.matmul(rps[:], ones_b[:], sq[:, 1, :],
                                 start=False, stop=True)
                srow = work.tile([1, TC], F32, tag="srow", name="srow")
                nc.scalar.activation(
                    srow[:], rps[:], AF.Sqrt,
                    bias=nbiasq[:] if tb == 0 else nbias1[:],
                    scale=1.0 if tb == 0 else 1.0 / H)
                rrow = work.tile([1, TC], F32, tag="rrow", name="rrow")
                nc.vector.reciprocal_approx_fast(rrow[:], srow[:])
                rrow_b = work.tile([1, TC], BF16, tag="rrowb", name="rrow_b")
                nc.vector.tensor_copy(rrow_b[:], rrow[:])
                rb = work.tile([128, TC], BF16, tag="rb", name="rb")
                nc.gpsimd.partition_broadcast(rb[:], rrow_b[:])
                ta = work.tile([128, TC], BF16, tag="wf", name="ta")
                nc.vector.tensor_tensor(ta[:], braw[:, 0, :],
                                        qtab_sb[:, tb + 0, :], OP.mult)
                tb_ = work.tile([128, TC], BF16, tag="wf", name="tb")
                nc.vector.tensor_tensor(tb_[:], braw[:, 1, :],
                                        qtab_sb[:, tb + 1, :], OP.mult)
                nc.vector.tensor_tensor(ta[:], ta[:], tb_[:], OP.subtract)
                nc.vector.tensor_tensor(dst0, ta[:], rb[:], OP.mult)
                ta2 = work.tile([128, TC], BF16, tag="wf", name="ta2")
                nc.vector.tensor_tensor(ta2[:], braw[:, 1, :],
                                        qtab_sb[:, tb + 2, :], OP.mult)
                tb2 = work.tile([128, TC], BF16, tag="wf", name="tb2")
                nc.vector.tensor_tensor(tb2[:], braw[:, 0, :],
                                        qtab_sb[:, tb + 3, :], OP.mult)
                nc.vector.tensor_tensor(ta2[:], ta2[:], tb2[:], OP.add)
                nc.vector.tensor_tensor(dst1, ta2[:], rb[:], OP.mult)

            # ---------------- phase A: K/V/Q projections + gathers ----------
            own0 = SW - TC
            with tc.tile_pool(name="wp", bufs=3) as wp, \
                 tc.tile_pool(name="work", bufs=2) as work, \
                 tc.tile_pool(name="ps12", bufs=2, space="PSUM") as ps12:
                pending = None
                for k in range(KV):
                    wk_sb = wp.tile([128, NDT, H], F8, tag="wh", name="wk")
                    nc.sync.dma_start(
                        wk_sb[:],
                        kwk[:, k * H:(k + 1) * H].rearrange(
                            "(dt p) h -> p dt h", p=128))
                    psp = ps12.tile([128, 2, TC], F32, tag="pspair", name="pspK")
                    for hh in range(2):
                        for j in range(NDT // 2):
                            nc.tensor.matmul(
                                psp[:, hh, :],
                                wk_sb[:, 2 * j:2 * j + 2, hh * 128:(hh + 1) * 128],
                                xq_pair(j),
                                start=(j == 0), stop=(j == NDT // 2 - 1),
                                perf_mode=DR)
                    if pending is not None:
                        pp, pk = pending
                        rope_norm_fold(pp, 4,
                                       kT_sb[:, pk * 2 + 0, own0:SW],
                                       kT_sb[:, pk * 2 + 1, own0:SW])
                    pending = (psp, k)
                pp, pk = pending
                rope_norm_fold(pp, 4,
                               kT_sb[:, pk * 2 + 0, own0:SW],
                               kT_sb[:, pk * 2 + 1, own0:SW])

                nc.gpsimd.dma_start(
                    klocal[:].rearrange("p (a b) -> p a b", a=NH),
                    kT_sb[:, :, own0:SW])
                nc.gpsimd.collective_compute(
                    "AllGather", OP.bypass,
                    replica_groups=[list(range(N_CORES))],
                    ins=[klocal[:]], outs=[kgath[:]],
                )
                pid = nc.gpsimd.partition_id()

                def v_epilogue(psv, k, st):
                    scr = work.tile([128, H], BF16, tag="vscr", name="vscr")
                    rv2 = work.tile([128, 1], F32, tag="rv2", name="rv2")
                    nc.scalar.activation(scr[:], psv[:], AF.Square,
                                         accum_out=rv2[:])
                    srv = work.tile([128, 1], F32, tag="srv", name="srv")
                    nc.scalar.activation(
                        srv[:], rv2[:], AF.Sqrt, bias=epsvm[:],
                        scale=1.0 / (H * V_SCALE * V_SCALE))
                    rv = work.tile([128, 1], F32, tag="rv", name="rv")
                    nc.vector.reciprocal_approx_fast(rv[:], srv[:])
                    nc.vector.tensor_scalar_mul(
                        V_sb[:, NST - NOT + st, k, :], psv[:], rv[:])

                pend_v = None
                for k in range(KV):
                    vw_sb = wp.tile([128, NDT, H], F8, tag="wh", name="vw")
                    nc.sync.dma_start(
                        vw_sb[:],
                        kwv[:, k * H:(k + 1) * H].rearrange(
                            "(dt p) h -> p dt h", p=128))
                    for st in range(NOT):
                        psv = ps12.tile([128, H], F32, tag="psv", name="psv")
                        for j in range(NDT // 2):
                            nc.tensor.matmul(
                                psv[:],
                                xq_pair(j)[:, :, st * 128:(st + 1) * 128],
                                vw_sb[:, 2 * j:2 * j + 2, :],
                                start=(j == 0), stop=(j == NDT // 2 - 1),
                                perf_mode=DR)
                        if pend_v is not None:
                            v_epilogue(*pend_v)
                        pend_v = (psv, k, st)
                v_epilogue(*pend_v)

                nc.gpsimd.dma_start(
                    vlocal[:].rearrange("p (a k c) -> p a k c", a=NOT, k=KV),
                    V_sb[:, NST - NOT:NST, :, :])
                nc.gpsimd.collective_compute(
                    "AllGather", OP.bypass,
                    replica_groups=[list(range(N_CORES))],
                    ins=[vlocal[:]], outs=[vgath[:]],
                )
                for j in range(2):
                    cj = ((pid + 6 + j) % N_CORES) * 128
                    nc.gpsimd.dma_start(
                        kT_sb[:, :, j * TC:(j + 1) * TC],
                        kgath[bass.ds(cj, 128), :].rearrange(
                            "p (a b) -> p a b", a=NH * 2))
                for j in range(2):
                    cj = ((pid + 6 + j) % N_CORES) * 128
                    nc.gpsimd.dma_start(
                        V_sb[:, NOT * j:NOT * (j + 1), :, :],
                        vgath[bass.ds(cj, 128), :].rearrange(
                            "p (a k c) -> p a k c", a=NOT, k=KV))

                # ------------- Q projections (overlap gather) -------
                pend_q = None
                for n in range(NH):
                    wq_sb = wp.tile([128, NDT, H], F8, tag="wh", name="wq")
                    nc.sync.dma_start(
                        wq_sb[:],
                        qw[:, n * H:(n + 1) * H].rearrange("(dt p) h -> p dt h", p=128))
                    psp = ps12.tile([128, 2, TC], F32, tag="pspair", name="pspQ")
                    for hh in range(2):
                        for j in range(NDT // 2):
                            nc.tensor.matmul(
                                psp[:, hh, :],
                                wq_sb[:, 2 * j:2 * j + 2, hh * 128:(hh + 1) * 128],
                                xq_pair(j),
                                start=(j == 0), stop=(j == NDT // 2 - 1),
                                perf_mode=DR)
                    if pend_q is not None:
                        pp, pn = pend_q
                        rope_norm_fold(pp, 0,
                                       qT_g[pn // 2][:, (pn % 2) * 2 + 0, :],
                                       qT_g[pn // 2][:, (pn % 2) * 2 + 1, :])
                    pend_q = (psp, n)
                pp, pn = pend_q
                rope_norm_fold(pp, 0,
                               qT_g[pn // 2][:, (pn % 2) * 2 + 0, :],
                               qT_g[pn // 2][:, (pn % 2) * 2 + 1, :])

            # ---------------- phase F: bf16 fixup of global rows 0..127 -----
            # All cores compute identical values from identical inputs; only
            # core 0 blends them in (fixmask).
            with tc.tile_pool(name="fw", bufs=1) as fw, \
                 tc.tile_pool(name="fwk", bufs=3) as fwk, \
                 tc.tile_pool(name="psF", bufs=1, space="PSUM") as psFp, \
                 tc.tile_pool(name="psFs", bufs=1, space="PSUM") as psFs:
                vF_sb = persist.tile([128, 8, 128], BF16)
                qn_e = fw.tile([128, 8, 128], BF16, tag="qne", name="qn_e")
                qn_o = fw.tile([128, 8, 128], BF16, tag="qno", name="qn_o")
                kn_e = fw.tile([128, 4, 128], BF16, tag="kne", name="kn_e")
                kn_o = fw.tile([128, 4, 128], BF16, tag="kno", name="kn_o")

                def fix_norm(ps_sl, dst_e, dst_o, vscale, qmode=False):
                    """ps_sl [128, 2, 128] raw f32 psum; normalized to bf16."""
                    sqo = fw.tile([128, 2, 128], BF16, tag="fsq", name="fsq", bufs=3)
                    acc = fw.tile([128, 1], F32, tag="facc", name="facc", bufs=3)
                    nc.scalar.activation(sqo[:], ps_sl, AF.Square,
                                         accum_out=acc[:])
                    sr = fw.tile([128, 1], F32, tag="fsr", name="fsr", bufs=3)
                    if qmode:
                        bias_t, sc = epsq128[:], 1.0
                    elif vscale != 1.0:
                        bias_t, sc = epsv[:], 1.0 / (H * vscale * vscale)
                    else:
                        bias_t, sc = eps1[:], 1.0 / H
                    nc.scalar.activation(sr[:], acc[:], AF.Sqrt,
                                         bias=bias_t, scale=sc)
                    rv = fw.tile([128, 1], F32, tag="frv", name="frv", bufs=3)
                    nc.vector.reciprocal_approx_fast(rv[:], sr[:])
                    nc.vector.tensor_scalar_mul(dst_e, ps_sl[:, 0, :], rv[:])
                    nc.vector.tensor_scalar_mul(dst_o, ps_sl[:, 1, :], rv[:])

                # pass 1: Q planes
                psFq = psFp.tile([128, 16, 128], F32, tag="psF", name="psFq")
                for dt in range(NDT):
                    wchunk = fwk.tile([128, 2048], BF16, tag="wfx", name="wfxq")
                    nc.sync.dma_start(
                        wchunk[:], wfixq[dt * 128:(dt + 1) * 128, :])
                    for c in range(4):
                        nc.tensor.matmul(
                            psFq[:, 4 * c:4 * c + 4, :], x16_sb[:, dt, :],
                            wchunk[:, c * 512:(c + 1) * 512],
                            start=(dt == 0), stop=(dt == NDT - 1))
                for n in range(NH):
                    fix_norm(psFq[:, 2 * n:2 * n + 2, :],
                             qn_e[:, n, :], qn_o[:, n, :], 1.0, qmode=True)
                # pass 2: K|V planes
                psFkv = psFp.tile([128, 16, 128], F32, tag="psF", name="psFkv")
                for dt in range(NDT):
                    wchunk = fwk.tile([128, 2048], BF16, tag="wfx", name="wfxkv")
                    nc.sync.dma_start(
                        wchunk[:], wfixkv[dt * 128:(dt + 1) * 128, :])
                    for c in range(4):
                        nc.tensor.matmul(
                            psFkv[:, 4 * c:4 * c + 4, :], x16_sb[:, dt, :],
                            wchunk[:, c * 512:(c + 1) * 512],
                            start=(dt == 0), stop=(dt == NDT - 1))
                for k in range(KV):
                    fix_norm(psFkv[:, 2 * k:2 * k + 2, :],
                             kn_e[:, k, :], kn_o[:, k, :], 1.0)
                for k in range(KV):
                    fix_norm(psFkv[:, 8 + 2 * k:8 + 2 * k + 2, :],
                             vF_sb[:, 2 * k, :], vF_sb[:, 2 * k + 1, :],
                             V_SCALE)

                def fix_rope(src_e, src_o, dst_e, dst_o, ft, nrep):
                    t1 = fw.tile([128, nrep, 128], BF16, tag="frp", name="f1", bufs=2)
                    nc.vector.tensor_tensor(t1[:], src_e[:], ft[:, 0, 0:nrep, :],
                                            OP.mult)
                    t2 = fw.tile([128, nrep, 128], BF16, tag="frp", name="f2", bufs=2)
                    nc.vector.tensor_tensor(t2[:], src_o[:], ft[:, 1, 0:nrep, :],
                                            OP.mult)
                    nc.vector.tensor_tensor(dst_e[:], t1[:], t2[:], OP.subtract)
                    t3 = fw.tile([128, nrep, 128], BF16, tag="frp", name="f3", bufs=2)
                    nc.vector.tensor_tensor(t3[:], src_o[:], ft[:, 2, 0:nrep, :],
                                            OP.mult)
                    t4 = fw.tile([128, nrep, 128], BF16, tag="frp", name="f4", bufs=2)
                    nc.vector.tensor_tensor(t4[:], src_e[:], ft[:, 3, 0:nrep, :],
                                            OP.mult)
                    nc.vector.tensor_tensor(dst_o[:], t3[:], t4[:], OP.add)

                qro_e = fw.tile([128, 8, 128], BF16, tag="qroe", name="qro_e")
                qro_o = fw.tile([128, 8, 128], BF16, tag="qroo", name="qro_o")
                kro_e = fw.tile([128, 4, 128], BF16, tag="kroe", name="kro_e")
                kro_o = fw.tile([128, 4, 128], BF16, tag="kroo", name="kro_o")
                fix_rope(qn_e, qn_o, qro_e, qro_o, ftq_sb, 8)
                fix_rope(kn_e, kn_o, kro_e, kro_o, ftk_sb, 4)

                qT_fix = persist.tile([128, 16, 128], BF16)
                kT_fix = persist.tile([128, 8, 128], BF16)
                for p in range(8):
                    for par, src in ((0, qro_e), (1, qro_o)):
                        pst = psFs.tile([128, 128], BF16, tag="ptr", name="ptr")
                        nc.tensor.transpose(pst[:], src[:, p, :], ident_sb[:])
                        nc.vector.tensor_copy(qT_fix[:, 2 * p + par, :], pst[:])
                for p in range(4):
                    for par, src in ((0, kro_e), (1, kro_o)):
                        pst = psFs.tile([128, 128], BF16, tag="ptr", name="ptr")
                        nc.tensor.transpose(pst[:], src[:, p, :], ident_sb[:])
                        nc.vector.tensor_copy(kT_fix[:, 2 * p + par, :], pst[:])

            # ---------------- phase B2: attention ----------------
            with tc.tile_pool(name="p3", bufs=1) as p3, \
                 tc.tile_pool(name="aw", bufs=4) as aw, \
                 tc.tile_pool(name="ps3", bufs=2, space="PSUM") as ps3, \
                 tc.tile_pool(name="psenc", bufs=2, space="PSUM") as psenc:
                maskT_sb = p3.tile([128, NST, TC], F32)       # 24 KB/p
                mr = maskT.rearrange("j p t -> p j t")
                nc.sync.dma_start(maskT_sb[:, 0:NST // 2, :], mr[:, 0:NST // 2, :])
                nc.sync.dma_start(maskT_sb[:, NST // 2:, :], mr[:, NST // 2:, :])

                for g in range(KV):
                    heads = (2 * g, 2 * g + 1)
                    encs = [psenc.tile([128, 2, TC], F32, tag="enc",
                                       name=f"enc{a}") for a in range(2)]
                    denb = ps3.tile([1, 2 * TC], F32, tag="den", name="denb",
                                    bufs=1)
                    def pv_step(pTpair, st):
                        # enc.T[h, t] += V.T @ P.T ; den[t] += sum_s P.T
                        for hh in range(2):
                            for a in range(2):
                                nc.tensor.matmul(
                                    encs[a][:, hh, :],
                                    V_sb[:, st, g, hh * 128:(hh + 1) * 128],
                                    pTpair[:, a, :],
                                    start=(st == 0), stop=(st == NST - 1))
                        for a in range(2):
                            nc.tensor.matmul(
                                denb[:, a * TC:(a + 1) * TC], ones_b[:],
                                pTpair[:, a, :],
                                start=(st == 0), stop=(st == NST - 1))

                    pend_pv = None
                    for st in range(NST):
                        psLs = [ps3.tile([128, TC], F32, tag="psL",
                                         name=f"psL{a}") for a in range(2)]
                        for hh in range(2):
                            for a, n in enumerate(heads):
                                nc.tensor.matmul(
                                    psLs[a][:],
                                    kT_sb[:, g * 2 + hh, st * 128:(st + 1) * 128],
                                    qT_g[g][:, a * 2 + hh, :],
                                    start=(hh == 0), stop=(hh == 1))
                        pTb = aw.tile([128, 2, TC], BF16, tag="pT", name="pTb",
                                      bufs=3)
                        pTs = [pTb[:, 0, :], pTb[:, 1, :]]
                        t1s = [aw.tile([128, TC], F32, tag="t1", name=f"t1{a}",
                                       bufs=4) for a in range(2)]
                        for a in range(2):
                            nc.scalar.activation(t1s[a][:], psLs[a][:], AF.Tanh,
                                                 scale=1.0 / SOFT_CAP)
                        for a in range(2):
                            nc.vector.tensor_tensor(t1s[a][:], t1s[a][:],
                                                    maskT_sb[:, st, :], OP.add)
                        for a in range(2):
                            nc.scalar.activation(pTs[a], t1s[a][:], AF.Exp,
                                                 scale=SOFT_CAP)
                        if pend_pv is not None:
                            pv_step(*pend_pv)
                        pend_pv = (pTb, st)
                    pv_step(*pend_pv)
                    for a, n in enumerate(heads):
                        drow = aw.tile([1, TC], F32, tag="drow", name="drow", bufs=2)
                        nc.vector.reciprocal_approx_fast(drow[:], denb[:, a * TC:(a + 1) * TC])
                        rbden = aw.tile([128, TC], F32, tag="rbden", name="rbden", bufs=2)
                        nc.gpsimd.partition_broadcast(rbden[:], drow[:])
                        for hh in range(2):
                            nc.vector.tensor_tensor(
                                encT_sb[:, n * 2 + hh, :], encs[a][:, hh, :],
                                rbden[:], OP.mult)
                            nc.vector.tensor_tensor(
                                enc0_16[:, n * 2 + hh, :],
                                encs[a][:, hh, 0:128],
                                rbden[:, 0:128], OP.mult)

            # ---------------- fixup attention (after B2) ----------------
            with tc.tile_pool(name="fw2", bufs=2) as fw2, \
                 tc.tile_pool(name="psF2", bufs=2, space="PSUM") as psF2:
                # fixup attention (single 128x128 causal tile)
                for g in range(KV):
                    psLf = psF2.tile([128, 2, 128], F32, tag="psLf", name="psLf")
                    for a in range(2):
                        for hh in range(2):
                            nc.tensor.matmul(
                                psLf[:, a, :],
                                kT_fix[:, g * 2 + hh, :],
                                qT_fix[:, (2 * g + a) * 2 + hh, :],
                                start=(hh == 0), stop=(hh == 1))
                    tf = fw2.tile([128, 2, 128], F32, tag="tf", name="tf")
                    nc.scalar.activation(tf[:], psLf[:], AF.Tanh,
                                         scale=1.0 / SOFT_CAP)
                    pf = fw2.tile([128, 2, 128], BF16, tag="pf", name="pf")
                    nc.scalar.activation(pf[:], tf[:], AF.Exp, scale=SOFT_CAP)
                    for a in range(2):
                        nc.vector.tensor_tensor(pf[:, a, :], pf[:, a, :],
                                                maskf_sb[:], OP.mult)
                    denf = psF2.tile([1, 2, 128], F32, tag="denf", name="denf")
                    encf = psF2.tile([128, 2, 2, 128], F32, tag="encf",
                                     name="encf")
                    for a in range(2):
                        nc.tensor.matmul(denf[:, a, :], ones_b[:], pf[:, a, :],
                                         start=True, stop=True)
                        for hh in range(2):
                            nc.tensor.matmul(
                                encf[:, a, hh, :],
                                vF_sb[:, g * 2 + hh, :],
                                pf[:, a, :],
                                start=True, stop=True)
                    for a in range(2):
                        drf = fw2.tile([1, 128], F32, tag="drf", name="drf")
                        nc.vector.reciprocal_approx_fast(drf[:], denf[:, a, :])
                        rbf = fw2.tile([128, 128], F32, tag="rbf", name="rbf")
                        nc.gpsimd.partition_broadcast(rbf[:], drf[:])
                        n = 2 * g + a
                        for hh in range(2):
                            nc.vector.tensor_tensor(
                                encF_16[:, n * 2 + hh, :], encf[:, a, hh, :],
                                rbf[:], OP.mult)

            # ---------------- blend + phase C: output projection ----------
            with tc.tile_pool(name="outp", bufs=2) as outp, \
                 tc.tile_pool(name="owp", bufs=2) as owp, \
                 tc.tile_pool(name="ps4", bufs=4, space="PSUM") as ps4:
                delta = outp.tile([128, NH * 2, 128], BF16, tag="dl",
                                  name="delta", bufs=1)
                nc.vector.tensor_tensor(delta[:], encF_16[:], enc0_16[:],
                                        OP.subtract)
                nc.vector.tensor_scalar_mul(delta[:], delta[:], fixm_sb[:])
                enc0f = persist.tile([128, NH * 2, 128], BF16)
                nc.vector.tensor_tensor(enc0f[:], enc0_16[:], delta[:], OP.add)

                for dc in range(4):
                    ow_sb = owp.tile([128, NH * 2, 512], F8, tag="ow",
                                     name="ow_sb")
                    nc.sync.dma_start(
                        ow_sb[:],
                        ow[:, dc * 512:(dc + 1) * 512].rearrange(
                            "(nh p) d -> p nh d", p=128))
                    ow16_sb = owp.tile([128, NH * 2, 512], BF16, tag="ow16",
                                       name="ow16_sb")
                    nc.sync.dma_start(
                        ow16_sb[:],
                        ow16[:, dc * 512:(dc + 1) * 512].rearrange(
                            "(nh p) d -> p nh d", p=128))
                    # t-tile 0: bf16 with blended encodings
                    psO = ps4.tile([128, 512], F32, tag="psO", name="psO0")
                    for p in range(NH * 2):
                        nc.tensor.matmul(
                            psO[:], enc0f[:, p, :], ow16_sb[:, p, :],
                            start=(p == 0), stop=(p == NH * 2 - 1))
                    ob = outp.tile([128, 512], F32, tag="ob", name="ob")
                    nc.vector.tensor_scalar(
                        ob[:], psO[:], 1.0 / V_SCALE, None, OP.mult)
                    nc.sync.dma_start(out[0:128, dc * 512:(dc + 1) * 512], ob[:])
                    # t-tiles 1..3: fp8 DoubleRow
                    for tt in range(1, NTT):
                        psO = ps4.tile([128, 512], F32, tag="psO", name="psO")
                        for p in range(NH):
                            nc.tensor.matmul(
                                psO[:],
                                encT_sb[:, 2 * p:2 * p + 2,
                                        tt * 128:(tt + 1) * 128],
                                ow_sb[:, 2 * p:2 * p + 2, :],
                                start=(p == 0), stop=(p == NH - 1),
                                perf_mode=DR)
                        ob = outp.tile([128, 512], F32, tag="ob", name="ob")
                        nc.vector.tensor_scalar(
                            ob[:], psO[:], 1.0 / (W_SCALE * V_SCALE), None,
                            OP.mult)
                        nc.sync.dma_start(
                            out[tt * 128:(tt + 1) * 128,
                                dc * 512:(dc + 1) * 512],
                            ob[:])

    nc.compile()
    return nc


_NC_CACHE = None


def _get_program():
    global _NC_CACHE
    if _NC_CACHE is None:
        _NC_CACHE = build_program()
    return _NC_CACHE


def prepare_inputs(x, q_w, kv_w, o_w, q_scale, k_scale, v_scale, segment_pos,
                   attn_mask):
    """Host-side prep: shard + transpose + fold scales + tables + masks."""
    x = np.asarray(x)
    q_w, kv_w, o_w = np.asarray(q_w), np.asarray(kv_w), np.asarray(o_w)
    q_scale, k_scale, v_scale = (np.asarray(q_scale), np.asarray(k_scale),
                                 np.asarray(v_scale))
    segment_pos = np.asarray(segment_pos)
    attn_mask = np.asarray(attn_mask)
    assert x.shape == (1, T, D)

    qs, ks, vs = 1.0 + q_scale, 1.0 + k_scale, 1.0 + v_scale
    # raw weights (scale vectors folded into rope tables / o_w instead)
    qw_flat = q_w.transpose(1, 0, 2).reshape(D, NH * H)
    kwk_flat = kv_w[0].transpose(1, 0, 2).reshape(D, KV * H)
    kwv_flat = kv_w[1].transpose(1, 0, 2).reshape(D, KV * H)
    ow_flat = o_w.reshape(NH, H, D) * vs[None, :, None]
    ow_flat = ow_flat.reshape(NH * H, D)
    f8 = ml_dtypes.float8_e4m3
    bf = ml_dtypes.bfloat16
    qw_b = np.ascontiguousarray(qw_flat * W_SCALE, dtype=f8)
    kwk_b = np.ascontiguousarray(kwk_flat * W_SCALE, dtype=f8)
    kwv_b = np.ascontiguousarray(kwv_flat * W_SCALE, dtype=f8)
    ow_b = np.ascontiguousarray(ow_flat * W_SCALE, dtype=f8)
    ow16_b = np.ascontiguousarray(ow_flat, dtype=bf)
    # fixup weight packs in bf16: q heads, then k|v heads
    wfixq_b = np.ascontiguousarray(qw_flat, dtype=bf)
    wfixkv_b = np.ascontiguousarray(
        np.concatenate([kwk_flat, kwv_flat], axis=1), dtype=bf)

    pos = segment_pos[0].astype(np.float64)
    freq = ROPE_BASE ** (2.0 * np.arange(HH) / H)
    xt_full = np.ascontiguousarray(x[0].T, dtype=f8)   # [D, T]
    x16_b = np.ascontiguousarray(x[0].T[:, 0:128], dtype=bf)
    am = attn_mask[0]                                  # [T, T] bool

    # fixup rope tables in [t, dim] layout for global positions 0..127,
    # replicated across head planes
    ang0 = pos[0:128, None] / freq[None, :]            # [128, HH]
    c0, s0 = np.cos(ang0), np.sin(ang0)

    def fixtabs(sc, nrep):
        t = np.stack([c0 * sc[None, :HH], s0 * sc[None, HH:],
                      c0 * sc[None, HH:], s0 * sc[None, :HH]])  # [4,128,HH]
        t = np.broadcast_to(t[:, :, None, :], (4, 128, nrep, HH))
        return np.ascontiguousarray(
            t.transpose(1, 0, 2, 3).reshape(128, 4 * nrep * HH), dtype=bf)

    ftq_b = fixtabs(qs, 8)
    ftk_b = fixtabs(ks, 4)
    maskf_b = np.ascontiguousarray(
        np.tril(np.ones((128, 128))).T, dtype=bf)      # [s, t] s<=t
    ident_b = np.ascontiguousarray(np.eye(128), dtype=bf)

    t_all = np.arange(T)
    in_maps = []
    for c in range(N_CORES):
        t_lo = c * TC
        xq_c = np.ascontiguousarray(xt_full[:, t_lo:t_lo + TC])

        ang = pos[t_lo:t_lo + TC][None, :] / freq[:, None]   # [HH, TC]
        cosq_c, sinq_c = np.cos(ang), np.sin(ang)
        # main-pass rope tables [plane, HH, TC]: Q(A1,A2,B1,B2), K(...)
        qtab_c = np.stack([
            cosq_c * qs[:HH, None], sinq_c * qs[HH:, None],
            cosq_c * qs[HH:, None], sinq_c * qs[:HH, None],
            cosq_c * ks[:HH, None], sinq_c * ks[HH:, None],
            cosq_c * ks[HH:, None], sinq_c * ks[:HH, None],
        ]).astype(bf)

        s_idx = np.arange(t_lo - WINDOW, t_lo + TC)    # [SW]
        valid_s = s_idx >= 0
        sv = s_idx[valid_s]
        t_g = t_all[t_lo:t_lo + TC]
        m = np.zeros((SW, TC), dtype=bool)
        m[valid_s] = am[t_lo:t_lo + TC][:, sv].T
        dwin = t_g[None, :] - s_idx[:, None]
        m &= (dwin >= 0) & (dwin < WINDOW)
        maskT_c = np.where(m, 0.0, -4.0).astype(np.float32).reshape(
            NST, 128, TC)

        fixm_c = np.full((128, 1), 1.0 if c == 0 else 0.0, dtype=np.float32)

        in_maps.append(dict(
            xq=xq_c, qw=qw_b, kwk=kwk_b, kwv=kwv_b, ow=ow_b, ow16=ow16_b,
            qtab=qtab_c, maskT=maskT_c, x16=x16_b, wfixq=wfixq_b,
            wfixkv=wfixkv_b, ftq=ftq_b, ftk=ftk_b, maskf=maskf_b,
            fixmask=fixm_c, ident=ident_b,
        ))
    return in_maps


def run(in_maps, trace=False, **kwargs):
    nc = _get_program()
    return run_bass_kernel_spmd(nc, in_maps, core_ids=list(range(N_CORES)),
                                trace=trace, **kwargs)


def kernel(**inputs) -> np.ndarray:
    in_maps = prepare_inputs(**inputs)
    res = run(in_maps)
    out = np.concatenate([res.results[c]["out"] for c in range(N_CORES)], axis=0)
    return out.reshape(1, T, D).astype(np.float32)


if __name__ == "__main__":
    nc = _get_program()
    print("built + compiled OK")


# revision 39
# speedup vs baseline: 1.2720x; 1.2720x over previous
"""Trainium2 Bass kernel for nn_Attention_28802050687686.

GQA sliding-window attention, T=4096, D=2048, 8 Q heads / 4 KV heads,
head_dim 256, window 1024, tanh soft-cap 50, RMSNorm+RoPE on Q/K, RMSNorm on V.

Sharding: sequence-parallel over 8 NeuronCores. Core c owns queries
[512c, 512c+512). Each core computes K/V for its OWN 512 rows only, then an
AllGather (via DRAM) distributes K/V; each core DMAs just its 1536-position
sliding window back into SBUF using partition-id-indexed dynamic offsets
(wrapped mod 8 -- out-of-range chunks land in fully-masked positions).
"""
import sys

sys.path.insert(0, "/opt/trn_rl_repo")

import numpy as np
import ml_dtypes

import concourse.bass as bass
import concourse.tile as tile
from concourse import bacc, mybir
from concourse.bass_utils import run_bass_kernel_spmd

F32 = mybir.dt.float32
BF16 = mybir.dt.bfloat16
AF = mybir.ActivationFunctionType
OP = mybir.AluOpType

# problem constants
T, D, NH, KV, H, HH = 4096, 2048, 8, 4, 256, 128
N_CORES = 8
TC = 512          # queries / own kv rows per core
SW = 1536         # kv window per core
NST = SW // 128   # 12 s-tiles in window
NOT = TC // 128   # 4 own s-tiles
NDT = D // 128    # 16 d-tiles
NTT = TC // 128   # 4 t-tiles
WINDOW = 1024
SOFT_CAP = 50.0
EPS = 1e-6
ROPE_BASE = 10000.0

KCOLS = NH * TC            # 4096 cols of K in the kv-local pack (8 htiles x 512)
VCOLS = NOT * KV * 256     # 4096 cols of V pack
KVCOLS = KCOLS + VCOLS     # 8192


def build_program():
    nc = bacc.Bacc("TRN2", target_bir_lowering=False, debug=False)

    xq = nc.dram_tensor("xq", [D, TC], BF16, kind="ExternalInput").ap()
    qw = nc.dram_tensor("qw", [D, NH * H], BF16, kind="ExternalInput").ap()
    kwk = nc.dram_tensor("kwk", [D, KV * H], BF16, kind="ExternalInput").ap()
    kwv = nc.dram_tensor("kwv", [D, KV * H], BF16, kind="ExternalInput").ap()
    ow = nc.dram_tensor("ow", [NH * H, D], BF16, kind="ExternalInput").ap()
    cosq = nc.dram_tensor("cosq", [HH, TC], F32, kind="ExternalInput").ap()
    sinq = nc.dram_tensor("sinq", [HH, TC], F32, kind="ExternalInput").ap()
    maskT = nc.dram_tensor("maskT", [NST, 128, TC], F32, kind="ExternalInput").ap()
    inv2q = nc.dram_tensor("inv2q", [HH, 2], BF16, kind="ExternalInput").ap()
    inv2k = nc.dram_tensor("inv2k", [HH, 2], BF16, kind="ExternalInput").ap()
    inv2v = nc.dram_tensor("inv2v", [1, KV * H], F32, kind="ExternalInput").ap()
    out = nc.dram_tensor("out", [TC, D], F32, kind="ExternalOutput").ap()

    klocal = nc.dram_tensor("klocal", [128, KCOLS], BF16).ap()
    kgath = nc.dram_tensor("kgath", [N_CORES * 128, KCOLS], BF16,
                           addr_space="Shared").ap()
    vlocal = nc.dram_tensor("vlocal", [128, VCOLS], BF16).ap()
    vgath = nc.dram_tensor("vgath", [N_CORES * 128, VCOLS], BF16,
                           addr_space="Shared").ap()

    with tile.TileContext(nc) as tc:
        with tc.tile_pool(name="persist", bufs=1) as persist, \
             tc.tile_pool(name="work", bufs=2) as work, \
             tc.tile_pool(name="owp", bufs=2) as owp:
            kT_sb = persist.tile([128, KV * 2, SW], BF16)     # 24 KB/p
            V_sb = persist.tile([128, NST, KV, 256], BF16)    # 24 KB/p
            qT_g = [persist.tile([128, 4, TC], BF16, name=f"qT{g}")
                    for g in range(KV)]                       # 16 KB/p total
            encT_sb = persist.tile([128, NH * 2, TC], BF16)   # 16 KB/p
            xq_ch = []
            for ch in range(4):
                xc = persist.tile([128, NDT // 4, TC], BF16, name=f"xq{ch}")
                nc.sync.dma_start(
                    xc[:], xq[ch * (D // 4):(ch + 1) * (D // 4), :].rearrange(
                        "(dt p) s -> p dt s", p=128))
                xq_ch.append(xc)

            def xq_sb(dt):
                return xq_ch[dt // 4][:, dt % 4, :]
            cosq_sb = persist.tile([HH, TC], F32)
            nc.sync.dma_start(cosq_sb[:], cosq[:])
            sinq_sb = persist.tile([HH, TC], F32)
            nc.sync.dma_start(sinq_sb[:], sinq[:])
            inv2q_sb = persist.tile([HH, 2], BF16)
            nc.sync.dma_start(inv2q_sb[:], inv2q[:])
            inv2k_sb = persist.tile([HH, 2], BF16)
            nc.sync.dma_start(inv2k_sb[:], inv2k[:])
            inv2v_sb = persist.tile([128, KV * H], F32)       # 4 KB/p
            nc.sync.dma_start(inv2v_sb[:], inv2v.to_broadcast([128, KV * H]))
            epsq1 = persist.tile([1, 1], F32)
            nc.vector.memset(epsq1[:], float(H) * EPS)
            epsk1 = persist.tile([1, 1], F32)
            nc.vector.memset(epsk1[:], EPS)
            eps128 = persist.tile([128, 1], F32)
            nc.vector.memset(eps128[:], EPS)
            ones_f = persist.tile([1, 128], BF16)
            nc.vector.memset(ones_f[:], 1.0)
            ones_b = persist.tile([128, 1], BF16)
            nc.vector.memset(ones_b[:], 1.0)

            def rope_norm_fold(ps_pair, inv2_sb, eps_t, dst0, dst1, bcast):
                """RMSNorm (exact via inv2 weights) + RoPE on an h-pair PSUM
                [128, 2, TC]; writes bf16 to dst0/dst1 [128, TC]."""
                sq0 = work.tile([128, TC], BF16, tag="wsq", name="sq0")
                nc.scalar.activation(sq0[:], ps_pair[:, 0, :], AF.Square)
                sq1 = work.tile([128, TC], BF16, tag="wsq", name="sq1")
                nc.scalar.activation(sq1[:], ps_pair[:, 1, :], AF.Square)
                rps = ps12.tile([1, TC], F32, tag="rowps", name="rps")
                nc.tensor.matmul(rps[:], inv2_sb[:, 0:1], sq0[:],
                                 start=True, stop=False)
                nc.tensor.matmul(rps[:], inv2_sb[:, 1:2], sq1[:],
                                 start=False, stop=True)
                srow = work.tile([1, TC], F32, tag="srow", name="srow")
                nc.scalar.activation(srow[:], rps[:], AF.Sqrt, bias=eps_t[:])
                rrow = work.tile([1, TC], F32, tag="rrow", name="rrow")
                nc.vector.reciprocal_approx_fast(rrow[:], srow[:])
                if bcast == "gpsimd":
                    rb = work.tile([128, TC], F32, tag="rb", name="rb")
                    nc.gpsimd.partition_broadcast(rb[:], rrow[:])
                else:
                    rrow_b = work.tile([1, TC], BF16, tag="rrowb", name="rrow_b")
                    nc.vector.tensor_copy(rrow_b[:], rrow[:])
                    rb = ps12.tile([128, TC], F32, tag="psv", name="rbps")
                    nc.tensor.matmul(rb[:], ones_f[:], rrow_b[:],
                                     start=True, stop=True)
                ta = work.tile([128, TC], F32, tag="wf", name="ta")
                nc.vector.tensor_tensor(ta[:], ps_pair[:, 0, :], cosq_sb[:], OP.mult)
                tb = work.tile([128, TC], F32, tag="wf", name="tb")
                nc.vector.tensor_tensor(tb[:], ps_pair[:, 1, :], sinq_sb[:], OP.mult)
                nc.vector.tensor_tensor(ta[:], ta[:], tb[:], OP.subtract)
                nc.vector.tensor_tensor(dst0, ta[:], rb[:], OP.mult)
                ta2 = work.tile([128, TC], F32, tag="wf", name="ta2")
                nc.vector.tensor_tensor(ta2[:], ps_pair[:, 1, :], cosq_sb[:], OP.mult)
                tb2 = work.tile([128, TC], F32, tag="wf", name="tb2")
                nc.vector.tensor_tensor(tb2[:], ps_pair[:, 0, :], sinq_sb[:], OP.mult)
                nc.vector.tensor_tensor(ta2[:], ta2[:], tb2[:], OP.add)
                nc.vector.tensor_tensor(dst1, ta2[:], rb[:], OP.mult)

            # ---------------- phase A: own-row K/V projections ----------------
            own0 = SW - TC  # own rows start at window col 1024
            with tc.tile_pool(name="wp", bufs=3) as wp, \
                 tc.tile_pool(name="ps12", bufs=2, space="PSUM") as ps12:
                pending = None
                for k in range(KV):
                    wk_sb = wp.tile([128, NDT, H], BF16, tag="wh", name="wk")
                    nc.sync.dma_start(
                        wk_sb[:],
                        kwk[:, k * H:(k + 1) * H].rearrange("(dt p) h -> p dt h", p=128))
                    psp = ps12.tile([128, 2, TC], F32, tag="pspair", name="pspK")
                    for hh in range(2):
                        for dt in range(NDT):
                            nc.tensor.matmul(
                                psp[:, hh, :],
                                wk_sb[:, dt, hh * 128:(hh + 1) * 128],
                                xq_sb(dt),
                                start=(dt == 0), stop=(dt == NDT - 1))
                    if pending is not None:
                        pp, pk = pending
                        rope_norm_fold(pp, inv2k_sb, epsk1,
                                       kT_sb[:, pk * 2 + 0, own0:SW],
                                       kT_sb[:, pk * 2 + 1, own0:SW], "gpsimd")
                    pending = (psp, k)
                pp, pk = pending
                rope_norm_fold(pp, inv2k_sb, epsk1,
                               kT_sb[:, pk * 2 + 0, own0:SW],
                               kT_sb[:, pk * 2 + 1, own0:SW], "gpsimd")

                nc.sync.dma_start(
                    klocal[:].rearrange("p (a b) -> p a b", a=NH),
                    kT_sb[:, :, own0:SW])
                nc.gpsimd.collective_compute(
                    "AllGather", OP.bypass,
                    replica_groups=[list(range(N_CORES))],
                    ins=[klocal[:]], outs=[kgath[:]],
                )
                pid = nc.gpsimd.partition_id()
                for j in range(2):
                    cj = ((pid + 6 + j) % N_CORES) * 128
                    nc.gpsimd.dma_start(
                        kT_sb[:, :, j * TC:(j + 1) * TC],
                        kgath[bass.ds(cj, 128), :].rearrange(
                            "p (a b) -> p a b", a=NH * 2))

                def v_epilogue(psv, k, st):
                    sqv = work.tile([128, H], F32, tag="sqv", name="sqv")
                    nc.scalar.activation(sqv[:], psv[:], AF.Square)
                    sqw = work.tile([128, H], F32, tag="sqw", name="sqw")
                    nc.vector.tensor_tensor(
                        sqw[:], sqv[:], inv2v_sb[:, k * H:(k + 1) * H], OP.mult)
                    rv2 = work.tile([128, 1], F32, tag="rv2", name="rv2")
                    nc.vector.tensor_reduce(rv2[:], sqw[:],
                                            mybir.AxisListType.X, OP.add)
                    srv = work.tile([128, 1], F32, tag="srv", name="srv")
                    nc.scalar.activation(srv[:], rv2[:], AF.Sqrt, bias=eps128[:])
                    rv = work.tile([128, 1], F32, tag="rv", name="rv")
                    nc.vector.reciprocal_approx_fast(rv[:], srv[:])
                    nc.vector.tensor_scalar_mul(
                        V_sb[:, NST - NOT + st, k, :], psv[:], rv[:])

                pend_v = None
                for k in range(KV):
                    vw_sb = wp.tile([128, NDT, H], BF16, tag="wh", name="vw")
                    nc.sync.dma_start(
                        vw_sb[:],
                        kwv[:, k * H:(k + 1) * H].rearrange("(dt p) h -> p dt h", p=128))
                    for st in range(NOT):
                        psv = ps12.tile([128, H], F32, tag="psv", name="psv")
                        for dt in range(NDT):
                            nc.tensor.matmul(
                                psv[:],
                                xq_sb(dt)[:, st * 128:(st + 1) * 128],
                                vw_sb[:, dt, :],
                                start=(dt == 0), stop=(dt == NDT - 1))
                        if pend_v is not None:
                            v_epilogue(*pend_v)
                        pend_v = (psv, k, st)
                v_epilogue(*pend_v)

                nc.sync.dma_start(
                    vlocal[:].rearrange("p (a k c) -> p a k c", a=NOT, k=KV),
                    V_sb[:, NST - NOT:NST, :, :])
                nc.gpsimd.collective_compute(
                    "AllGather", OP.bypass,
                    replica_groups=[list(range(N_CORES))],
                    ins=[vlocal[:]], outs=[vgath[:]],
                )
                for j in range(2):
                    cj = ((pid + 6 + j) % N_CORES) * 128
                    nc.gpsimd.dma_start(
                        V_sb[:, NOT * j:NOT * (j + 1), :, :],
                        vgath[bass.ds(cj, 128), :].rearrange(
                            "p (a k c) -> p a k c", a=NOT, k=KV))

                # ------------- phase B1: Q projections (overlap gather) -------
                pend_q = None
                for n in range(NH):
                    wq_sb = wp.tile([128, NDT, H], BF16, tag="wh", name="wq")
                    nc.sync.dma_start(
                        wq_sb[:],
                        qw[:, n * H:(n + 1) * H].rearrange("(dt p) h -> p dt h", p=128))
                    psp = ps12.tile([128, 2, TC], F32, tag="pspair", name="pspQ")
                    for hh in range(2):
                        for dt in range(NDT):
                            nc.tensor.matmul(
                                psp[:, hh, :],
                                wq_sb[:, dt, hh * 128:(hh + 1) * 128],
                                xq_sb(dt),
                                start=(dt == 0), stop=(dt == NDT - 1))
                    if pend_q is not None:
                        pp, pn = pend_q
                        rope_norm_fold(pp, inv2q_sb, epsq1,
                                       qT_g[pn // 2][:, (pn % 2) * 2 + 0, :],
                                       qT_g[pn // 2][:, (pn % 2) * 2 + 1, :], "pe")
                    pend_q = (psp, n)
                pp, pn = pend_q
                rope_norm_fold(pp, inv2q_sb, epsq1,
                               qT_g[pn // 2][:, (pn % 2) * 2 + 0, :],
                               qT_g[pn // 2][:, (pn % 2) * 2 + 1, :], "pe")


            # ---------------- phase B2: attention ----------------
            with tc.tile_pool(name="p3", bufs=1) as p3, \
                 tc.tile_pool(name="aw", bufs=4) as aw, \
                 tc.tile_pool(name="ps3", bufs=2, space="PSUM") as ps3, \
                 tc.tile_pool(name="psenc", bufs=2, space="PSUM") as psenc:
                maskT_sb = p3.tile([128, NST, TC], F32)       # 24 KB/p
                mr = maskT.rearrange("j p t -> p j t")
                nc.sync.dma_start(maskT_sb[:, 0:NST // 2, :], mr[:, 0:NST // 2, :])
                nc.sync.dma_start(maskT_sb[:, NST // 2:, :], mr[:, NST // 2:, :])

                for g in range(KV):
                    heads = (2 * g, 2 * g + 1)
                    encs = [psenc.tile([128, 2, TC], F32, tag="enc",
                                       name=f"enc{a}") for a in range(2)]
                    denb = ps3.tile([1, 2 * TC], F32, tag="den", name="denb",
                                    bufs=1)
                    def pv_step(pTpair, st):
                        # enc.T[h, t] += V.T @ P.T ; den[t] += sum_s P.T
                        for hh in range(2):
                            for a in range(2):
                                nc.tensor.matmul(
                                    encs[a][:, hh, :],
                                    V_sb[:, st, g, hh * 128:(hh + 1) * 128],
                                    pTpair[:, a, :],
                                    start=(st == 0), stop=(st == NST - 1))
                        for a in range(2):
                            nc.tensor.matmul(
                                denb[:, a * TC:(a + 1) * TC], ones_b[:],
                                pTpair[:, a, :],
                                start=(st == 0), stop=(st == NST - 1))

                    pend_pv = None
                    for st in range(NST):
                        psLs = [ps3.tile([128, TC], F32, tag="psL",
                                         name=f"psL{a}") for a in range(2)]
                        for hh in range(2):
                            for a, n in enumerate(heads):
                                nc.tensor.matmul(
                                    psLs[a][:],
                                    kT_sb[:, g * 2 + hh, st * 128:(st + 1) * 128],
                                    qT_g[g][:, a * 2 + hh, :],
                                    start=(hh == 0), stop=(hh == 1))
                        pTb = aw.tile([128, 2, TC], BF16, tag="pT", name="pTb",
                                      bufs=3)
                        pTs = [pTb[:, 0, :], pTb[:, 1, :]]
                        t1s = [aw.tile([128, TC], F32, tag="t1", name=f"t1{a}",
                                       bufs=4) for a in range(2)]
                        for a in range(2):
                            nc.scalar.activation(t1s[a][:], psLs[a][:], AF.Tanh,
                                                 scale=1.0 / SOFT_CAP)
                        for a in range(2):
                            nc.vector.tensor_tensor(t1s[a][:], t1s[a][:],
                                                    maskT_sb[:, st, :], OP.add)
                        for a in range(2):
                            nc.scalar.activation(pTs[a], t1s[a][:], AF.Exp,
                                                 scale=SOFT_CAP)
                        if pend_pv is not None:
                            pv_step(*pend_pv)
                        pend_pv = (pTb, st)
                    pv_step(*pend_pv)
                    for a, n in enumerate(heads):
                        drow = aw.tile([1, TC], F32, tag="drow", name="drow", bufs=2)
                        nc.vector.reciprocal_approx_fast(drow[:], denb[:, a * TC:(a + 1) * TC])
                        rbden = aw.tile([128, TC], F32, tag="rbden", name="rbden", bufs=2)
                        nc.gpsimd.partition_broadcast(rbden[:], drow[:])
                        for hh in range(2):
                            nc.vector.tensor_tensor(
                                encT_sb[:, n * 2 + hh, :], encs[a][:, hh, :],
                                rbden[:], OP.mult)

            # ---------------- phase C: output projection ----------------
            with tc.tile_pool(name="outp", bufs=3) as outp, \
                 tc.tile_pool(name="ps4", bufs=4, space="PSUM") as ps4:
                for dc in range(4):
                    ow_sb = owp.tile([128, NH * 2, 512], BF16, tag="ow", name="ow_sb")
                    nc.sync.dma_start(
                        ow_sb[:],
                        ow[:, dc * 512:(dc + 1) * 512].rearrange(
                            "(nh p) d -> p nh d", p=128))
                    for tt in range(NTT):
                        psO = ps4.tile([128, 512], F32, tag="psO", name="psO")
                        for nh in range(NH * 2):
                            nc.tensor.matmul(
                                psO[:],
                                encT_sb[:, nh, tt * 128:(tt + 1) * 128],
                                ow_sb[:, nh, :],
                                start=(nh == 0), stop=(nh == NH * 2 - 1))
                        ob = outp.tile([128, 512], F32, tag="ob", name="ob")
                        nc.vector.tensor_copy(ob[:], psO[:])
                        nc.sync.dma_start(
                            out[tt * 128:(tt + 1) * 128, dc * 512:(dc + 1) * 512],
                            ob[:])

    nc.compile()
    return nc


_NC_CACHE = None


def _get_program():
    global _NC_CACHE
    if _NC_CACHE is None:
        _NC_CACHE = build_program()
    return _NC_CACHE


def prepare_inputs(x, q_w, kv_w, o_w, q_scale, k_scale, v_scale, segment_pos,
                   attn_mask):
    """Host-side prep: shard + transpose + fold scales + tables + masks."""
    x = np.asarray(x)
    q_w, kv_w, o_w = np.asarray(q_w), np.asarray(kv_w), np.asarray(o_w)
    q_scale, k_scale, v_scale = (np.asarray(q_scale), np.asarray(k_scale),
                                 np.asarray(v_scale))
    segment_pos = np.asarray(segment_pos)
    attn_mask = np.asarray(attn_mask)
    assert x.shape == (1, T, D)

    qs, ks, vs = 1.0 + q_scale, 1.0 + k_scale, 1.0 + v_scale
    qw_flat = (q_w * qs[None, None, :]).transpose(1, 0, 2).reshape(D, NH * H)
    kwk_flat = (kv_w[0] * ks[None, None, :]).transpose(1, 0, 2).reshape(D, KV * H)
    kwv_flat = (kv_w[1] * vs[None, None, :]).transpose(1, 0, 2).reshape(D, KV * H)
    ow_flat = o_w.reshape(NH * H, D)
    bf = ml_dtypes.bfloat16
    qw_b = np.ascontiguousarray(qw_flat, dtype=bf)
    kwk_b = np.ascontiguousarray(kwk_flat, dtype=bf)
    kwv_b = np.ascontiguousarray(kwv_flat, dtype=bf)
    ow_b = np.ascontiguousarray(ow_flat, dtype=bf)

    inv2q_arr = (qs ** -2.0).reshape(2, HH).T.astype(ml_dtypes.bfloat16)
    inv2k_arr = ((ks ** -2.0) / H).reshape(2, HH).T.astype(ml_dtypes.bfloat16)
    inv2v_arr = (np.tile(vs ** -2.0, KV) / H)[None, :].astype(np.float32)

    pos = segment_pos[0].astype(np.float64)
    freq = ROPE_BASE ** (2.0 * np.arange(HH) / H)
    xt_full = np.ascontiguousarray(x[0].T, dtype=bf)   # [D, T]
    am = attn_mask[0]                                  # [T, T] bool

    t_all = np.arange(T)
    in_maps = []
    for c in range(N_CORES):
        t_lo = c * TC
        xq_c = np.ascontiguousarray(xt_full[:, t_lo:t_lo + TC])

        ang = pos[t_lo:t_lo + TC][None, :] / freq[:, None]   # [HH, TC]
        cosq_c = np.cos(ang).astype(np.float32)
        sinq_c = np.sin(ang).astype(np.float32)

        s_idx = np.arange(t_lo - WINDOW, t_lo + TC)    # [SW]
        valid_s = s_idx >= 0
        sv = s_idx[valid_s]
        t_g = t_all[t_lo:t_lo + TC]
        m = np.zeros((SW, TC), dtype=bool)
        m[valid_s] = am[t_lo:t_lo + TC][:, sv].T
        dwin = t_g[None, :] - s_idx[:, None]
        m &= (dwin >= 0) & (dwin < WINDOW)
        maskT_c = np.where(m, 0.0, -4.0).astype(np.float32).reshape(NST, 128, TC)

        in_maps.append(dict(
            xq=xq_c, qw=qw_b, kwk=kwk_b, kwv=kwv_b, ow=ow_b,
            cosq=cosq_c, sinq=sinq_c, maskT=maskT_c,
            inv2q=inv2q_arr, inv2k=inv2k_arr, inv2v=inv2v_arr,
        ))
    return in_maps


def run(in_maps, trace=False, **kwargs):
    nc = _get_program()
    return run_bass_kernel_spmd(nc, in_maps, core_ids=list(range(N_CORES)),
                                trace=trace, **kwargs)


def kernel(**inputs) -> np.ndarray:
    in_maps = prepare_inputs(**inputs)
    res = run(in_maps)
    out = np.concatenate([res.results[c]["out"] for c in range(N_CORES)], axis=0)
    return out.reshape(1, T, D).astype(np.float32)


if __name__ == "__main__":
    nc = _get_program()
    print("built + compiled OK")


# revision 40
# speedup vs baseline: 1.3374x; 1.0513x over previous
"""Trainium2 Bass kernel for nn_Attention_28802050687686.

GQA sliding-window attention, T=4096, D=2048, 8 Q heads / 4 KV heads,
head_dim 256, window 1024, tanh soft-cap 50, RMSNorm+RoPE on Q/K, RMSNorm on V.

Sharding: sequence-parallel over 8 NeuronCores. Core c owns queries
[512c, 512c+512). Each core computes K/V for its OWN 512 rows only, then an
AllGather (via DRAM) distributes K/V; each core DMAs just its 1536-position
sliding window back into SBUF using partition-id-indexed dynamic offsets
(wrapped mod 8 -- out-of-range chunks land in fully-masked positions).
"""
import sys

sys.path.insert(0, "/opt/trn_rl_repo")

import numpy as np
import ml_dtypes

import concourse.bass as bass
import concourse.tile as tile
from concourse import bacc, mybir
from concourse.bass_utils import run_bass_kernel_spmd

F32 = mybir.dt.float32
BF16 = mybir.dt.bfloat16
AF = mybir.ActivationFunctionType
OP = mybir.AluOpType

# problem constants
T, D, NH, KV, H, HH = 4096, 2048, 8, 4, 256, 128
N_CORES = 8
TC = 512          # queries / own kv rows per core
SW = 1536         # kv window per core
NST = SW // 128   # 12 s-tiles in window
NOT = TC // 128   # 4 own s-tiles
NDT = D // 128    # 16 d-tiles
NTT = TC // 128   # 4 t-tiles
WINDOW = 1024
SOFT_CAP = 50.0
EPS = 1e-6
ROPE_BASE = 10000.0

KCOLS = NH * TC            # 4096 cols of K in the kv-local pack (8 htiles x 512)
VCOLS = NOT * KV * 256     # 4096 cols of V pack
KVCOLS = KCOLS + VCOLS     # 8192


def build_program():
    nc = bacc.Bacc("TRN2", target_bir_lowering=False, debug=False)

    xq = nc.dram_tensor("xq", [D, TC], BF16, kind="ExternalInput").ap()
    qw = nc.dram_tensor("qw", [D, NH * H], BF16, kind="ExternalInput").ap()
    kwk = nc.dram_tensor("kwk", [D, KV * H], BF16, kind="ExternalInput").ap()
    kwv = nc.dram_tensor("kwv", [D, KV * H], BF16, kind="ExternalInput").ap()
    ow = nc.dram_tensor("ow", [NH * H, D], BF16, kind="ExternalInput").ap()
    cosq = nc.dram_tensor("cosq", [HH, TC], F32, kind="ExternalInput").ap()
    sinq = nc.dram_tensor("sinq", [HH, TC], F32, kind="ExternalInput").ap()
    maskT = nc.dram_tensor("maskT", [NST, 128, TC], F32, kind="ExternalInput").ap()
    inv2q = nc.dram_tensor("inv2q", [HH, 2], BF16, kind="ExternalInput").ap()
    inv2k = nc.dram_tensor("inv2k", [HH, 2], BF16, kind="ExternalInput").ap()
    inv2v = nc.dram_tensor("inv2v", [1, KV * H], F32, kind="ExternalInput").ap()
    out = nc.dram_tensor("out", [TC, D], F32, kind="ExternalOutput").ap()

    klocal = nc.dram_tensor("klocal", [128, KCOLS], BF16).ap()
    kgath = nc.dram_tensor("kgath", [N_CORES * 128, KCOLS], BF16,
                           addr_space="Shared").ap()
    vlocal = nc.dram_tensor("vlocal", [128, VCOLS], BF16).ap()
    vgath = nc.dram_tensor("vgath", [N_CORES * 128, VCOLS], BF16,
                           addr_space="Shared").ap()

    with tile.TileContext(nc) as tc:
        with tc.tile_pool(name="persist", bufs=1) as persist, \
             tc.tile_pool(name="work", bufs=2) as work, \
             tc.tile_pool(name="owp", bufs=2) as owp:
            kT_own = persist.tile([128, KV * 2, TC], BF16)    # 8 KB/p
            kT_rem = persist.tile([128, KV * 2, 2 * TC], BF16)  # 16 KB/p
            V_own = persist.tile([128, NOT, KV, 256], BF16)   # 8 KB/p
            V_rem = persist.tile([128, 2 * NOT, KV, 256], BF16)  # 16 KB/p
            qT_g = [persist.tile([128, 4, TC], BF16, name=f"qT{g}")
                    for g in range(KV)]                       # 16 KB/p total
            encT_sb = persist.tile([128, NH * 2, TC], BF16)   # 16 KB/p
            xq_ch = []
            for ch in range(4):
                xc = persist.tile([128, NDT // 4, TC], BF16, name=f"xq{ch}")
                nc.sync.dma_start(
                    xc[:], xq[ch * (D // 4):(ch + 1) * (D // 4), :].rearrange(
                        "(dt p) s -> p dt s", p=128))
                xq_ch.append(xc)

            def xq_sb(dt):
                return xq_ch[dt // 4][:, dt % 4, :]
            cosq_sb = persist.tile([HH, TC], F32)
            nc.sync.dma_start(cosq_sb[:], cosq[:])
            sinq_sb = persist.tile([HH, TC], F32)
            nc.sync.dma_start(sinq_sb[:], sinq[:])
            inv2q_sb = persist.tile([HH, 2], BF16)
            nc.sync.dma_start(inv2q_sb[:], inv2q[:])
            inv2k_sb = persist.tile([HH, 2], BF16)
            nc.sync.dma_start(inv2k_sb[:], inv2k[:])
            inv2v_sb = persist.tile([128, KV * H], F32)       # 4 KB/p
            nc.sync.dma_start(inv2v_sb[:], inv2v.to_broadcast([128, KV * H]))
            epsq1 = persist.tile([1, 1], F32)
            nc.vector.memset(epsq1[:], float(H) * EPS)
            epsk1 = persist.tile([1, 1], F32)
            nc.vector.memset(epsk1[:], EPS)
            eps128 = persist.tile([128, 1], F32)
            nc.vector.memset(eps128[:], EPS)
            ones_f = persist.tile([1, 128], BF16)
            nc.vector.memset(ones_f[:], 1.0)
            ones_b = persist.tile([128, 1], BF16)
            nc.vector.memset(ones_b[:], 1.0)

            def rope_norm_fold(ps_pair, inv2_sb, eps_t, dst0, dst1, bcast):
                """RMSNorm (exact via inv2 weights) + RoPE on an h-pair PSUM
                [128, 2, TC]; writes bf16 to dst0/dst1 [128, TC]."""
                sq0 = work.tile([128, TC], BF16, tag="wsq", name="sq0")
                nc.scalar.activation(sq0[:], ps_pair[:, 0, :], AF.Square)
                sq1 = work.tile([128, TC], BF16, tag="wsq", name="sq1")
                nc.scalar.activation(sq1[:], ps_pair[:, 1, :], AF.Square)
                rps = ps12.tile([1, TC], F32, tag="rowps", name="rps")
                nc.tensor.matmul(rps[:], inv2_sb[:, 0:1], sq0[:],
                                 start=True, stop=False)
                nc.tensor.matmul(rps[:], inv2_sb[:, 1:2], sq1[:],
                                 start=False, stop=True)
                srow = work.tile([1, TC], F32, tag="srow", name="srow")
                nc.scalar.activation(srow[:], rps[:], AF.Sqrt, bias=eps_t[:])
                rrow = work.tile([1, TC], F32, tag="rrow", name="rrow")
                nc.vector.reciprocal_approx_fast(rrow[:], srow[:])
                if bcast == "gpsimd":
                    rb = work.tile([128, TC], F32, tag="rb", name="rb")
                    nc.gpsimd.partition_broadcast(rb[:], rrow[:])
                else:
                    rrow_b = work.tile([1, TC], BF16, tag="rrowb", name="rrow_b")
                    nc.vector.tensor_copy(rrow_b[:], rrow[:])
                    rb = ps12.tile([128, TC], F32, tag="psv", name="rbps")
                    nc.tensor.matmul(rb[:], ones_f[:], rrow_b[:],
                                     start=True, stop=True)
                ta = work.tile([128, TC], F32, tag="wf", name="ta")
                nc.vector.tensor_tensor(ta[:], ps_pair[:, 0, :], cosq_sb[:], OP.mult)
                tb = work.tile([128, TC], F32, tag="wf", name="tb")
                nc.vector.tensor_tensor(tb[:], ps_pair[:, 1, :], sinq_sb[:], OP.mult)
                nc.vector.tensor_tensor(ta[:], ta[:], tb[:], OP.subtract)
                nc.vector.tensor_tensor(dst0, ta[:], rb[:], OP.mult)
                ta2 = work.tile([128, TC], F32, tag="wf", name="ta2")
                nc.vector.tensor_tensor(ta2[:], ps_pair[:, 1, :], cosq_sb[:], OP.mult)
                tb2 = work.tile([128, TC], F32, tag="wf", name="tb2")
                nc.vector.tensor_tensor(tb2[:], ps_pair[:, 0, :], sinq_sb[:], OP.mult)
                nc.vector.tensor_tensor(ta2[:], ta2[:], tb2[:], OP.add)
                nc.vector.tensor_tensor(dst1, ta2[:], rb[:], OP.mult)

            # ---------------- phase A: own-row K/V projections ----------------
            own0 = SW - TC  # own rows start at window col 1024
            with tc.tile_pool(name="wp", bufs=3) as wp, \
                 tc.tile_pool(name="ps12", bufs=2, space="PSUM") as ps12:
                pending = None
                for k in range(KV):
                    wk_sb = wp.tile([128, NDT, H], BF16, tag="wh", name="wk")
                    nc.sync.dma_start(
                        wk_sb[:],
                        kwk[:, k * H:(k + 1) * H].rearrange("(dt p) h -> p dt h", p=128))
                    psp = ps12.tile([128, 2, TC], F32, tag="pspair", name="pspK")
                    for hh in range(2):
                        for dt in range(NDT):
                            nc.tensor.matmul(
                                psp[:, hh, :],
                                wk_sb[:, dt, hh * 128:(hh + 1) * 128],
                                xq_sb(dt),
                                start=(dt == 0), stop=(dt == NDT - 1))
                    if pending is not None:
                        pp, pk = pending
                        rope_norm_fold(pp, inv2k_sb, epsk1,
                                       kT_own[:, pk * 2 + 0, :],
                                       kT_own[:, pk * 2 + 1, :], "gpsimd")
                    pending = (psp, k)
                pp, pk = pending
                rope_norm_fold(pp, inv2k_sb, epsk1,
                               kT_own[:, pk * 2 + 0, :],
                               kT_own[:, pk * 2 + 1, :], "gpsimd")

                nc.gpsimd.dma_start(
                    klocal[:].rearrange("p (a b) -> p a b", a=NH),
                    kT_own[:])
                nc.gpsimd.collective_compute(
                    "AllGather", OP.bypass,
                    replica_groups=[list(range(N_CORES))],
                    ins=[klocal[:]], outs=[kgath[:]],
                )
                pid = nc.gpsimd.partition_id()

                def v_epilogue(psv, k, st):
                    sqv = work.tile([128, H], F32, tag="sqv", name="sqv")
                    nc.scalar.activation(sqv[:], psv[:], AF.Square)
                    sqw = work.tile([128, H], F32, tag="sqw", name="sqw")
                    nc.vector.tensor_tensor(
                        sqw[:], sqv[:], inv2v_sb[:, k * H:(k + 1) * H], OP.mult)
                    rv2 = work.tile([128, 1], F32, tag="rv2", name="rv2")
                    nc.vector.tensor_reduce(rv2[:], sqw[:],
                                            mybir.AxisListType.X, OP.add)
                    srv = work.tile([128, 1], F32, tag="srv", name="srv")
                    nc.scalar.activation(srv[:], rv2[:], AF.Sqrt, bias=eps128[:])
                    rv = work.tile([128, 1], F32, tag="rv", name="rv")
                    nc.vector.reciprocal_approx_fast(rv[:], srv[:])
                    nc.vector.tensor_scalar_mul(
                        V_own[:, st, k, :], psv[:], rv[:])

                pend_v = None
                for k in range(KV):
                    vw_sb = wp.tile([128, NDT, H], BF16, tag="wh", name="vw")
                    nc.sync.dma_start(
                        vw_sb[:],
                        kwv[:, k * H:(k + 1) * H].rearrange("(dt p) h -> p dt h", p=128))
                    for st in range(NOT):
                        psv = ps12.tile([128, H], F32, tag="psv", name="psv")
                        for dt in range(NDT):
                            nc.tensor.matmul(
                                psv[:],
                                xq_sb(dt)[:, st * 128:(st + 1) * 128],
                                vw_sb[:, dt, :],
                                start=(dt == 0), stop=(dt == NDT - 1))
                        if pend_v is not None:
                            v_epilogue(*pend_v)
                        pend_v = (psv, k, st)
                v_epilogue(*pend_v)

                nc.gpsimd.dma_start(
                    vlocal[:].rearrange("p (a k c) -> p a k c", a=NOT, k=KV),
                    V_own[:])
                nc.gpsimd.collective_compute(
                    "AllGather", OP.bypass,
                    replica_groups=[list(range(N_CORES))],
                    ins=[vlocal[:]], outs=[vgath[:]],
                )
                for j in range(2):
                    cj = ((pid + 6 + j) % N_CORES) * 128
                    nc.gpsimd.dma_start(
                        kT_rem[:, :, j * TC:(j + 1) * TC],
                        kgath[bass.ds(cj, 128), :].rearrange(
                            "p (a b) -> p a b", a=NH * 2))
                for j in range(2):
                    cj = ((pid + 6 + j) % N_CORES) * 128
                    nc.gpsimd.dma_start(
                        V_rem[:, NOT * j:NOT * (j + 1), :, :],
                        vgath[bass.ds(cj, 128), :].rearrange(
                            "p (a k c) -> p a k c", a=NOT, k=KV))

                # ------------- phase B1: Q projections (overlap gather) -------
                pend_q = None
                for n in range(NH):
                    wq_sb = wp.tile([128, NDT, H], BF16, tag="wh", name="wq")
                    nc.sync.dma_start(
                        wq_sb[:],
                        qw[:, n * H:(n + 1) * H].rearrange("(dt p) h -> p dt h", p=128))
                    psp = ps12.tile([128, 2, TC], F32, tag="pspair", name="pspQ")
                    for hh in range(2):
                        for dt in range(NDT):
                            nc.tensor.matmul(
                                psp[:, hh, :],
                                wq_sb[:, dt, hh * 128:(hh + 1) * 128],
                                xq_sb(dt),
                                start=(dt == 0), stop=(dt == NDT - 1))
                    if pend_q is not None:
                        pp, pn = pend_q
                        rope_norm_fold(pp, inv2q_sb, epsq1,
                                       qT_g[pn // 2][:, (pn % 2) * 2 + 0, :],
                                       qT_g[pn // 2][:, (pn % 2) * 2 + 1, :], "pe")
                    pend_q = (psp, n)
                pp, pn = pend_q
                rope_norm_fold(pp, inv2q_sb, epsq1,
                               qT_g[pn // 2][:, (pn % 2) * 2 + 0, :],
                               qT_g[pn // 2][:, (pn % 2) * 2 + 1, :], "pe")


            # ---------------- phase B2: attention ----------------
            with tc.tile_pool(name="p3", bufs=1) as p3, \
                 tc.tile_pool(name="aw", bufs=4) as aw, \
                 tc.tile_pool(name="ps3", bufs=2, space="PSUM") as ps3, \
                 tc.tile_pool(name="psenc", bufs=2, space="PSUM") as psenc:
                maskT_sb = p3.tile([128, NST, TC], F32)       # 24 KB/p
                mr = maskT.rearrange("j p t -> p j t")
                nc.sync.dma_start(maskT_sb[:, 0:NST // 2, :], mr[:, 0:NST // 2, :])
                nc.sync.dma_start(maskT_sb[:, NST // 2:, :], mr[:, NST // 2:, :])

                for g in range(KV):
                    heads = (2 * g, 2 * g + 1)
                    encs = [psenc.tile([128, 2, TC], F32, tag="enc",
                                       name=f"enc{a}") for a in range(2)]
                    denb = ps3.tile([1, 2 * TC], F32, tag="den", name="denb",
                                    bufs=1)
                    def pv_step(pTpair, st):
                        # enc.T[h, t] += V.T @ P.T ; den[t] += sum_s P.T
                        vsl = (V_own[:, st - 8, g, :] if st >= 8
                               else V_rem[:, st, g, :])
                        for hh in range(2):
                            for a in range(2):
                                nc.tensor.matmul(
                                    encs[a][:, hh, :],
                                    vsl[:, hh * 128:(hh + 1) * 128],
                                    pTpair[:, a, :],
                                    start=(st == 8), stop=(st == 7))
                        for a in range(2):
                            nc.tensor.matmul(
                                denb[:, a * TC:(a + 1) * TC], ones_b[:],
                                pTpair[:, a, :],
                                start=(st == 8), stop=(st == 7))

                    pend_pv = None
                    for st in [8, 9, 10, 11, 0, 1, 2, 3, 4, 5, 6, 7]:
                        if st >= 8:
                            ksl = kT_own[:, :, (st - 8) * 128:(st - 7) * 128]
                        else:
                            ksl = kT_rem[:, :, st * 128:(st + 1) * 128]
                        psLs = [ps3.tile([128, TC], F32, tag="psL",
                                         name=f"psL{a}") for a in range(2)]
                        for hh in range(2):
                            for a, n in enumerate(heads):
                                nc.tensor.matmul(
                                    psLs[a][:],
                                    ksl[:, g * 2 + hh, :],
                                    qT_g[g][:, a * 2 + hh, :],
                                    start=(hh == 0), stop=(hh == 1))
                        pTb = aw.tile([128, 2, TC], BF16, tag="pT", name="pTb",
                                      bufs=3)
                        pTs = [pTb[:, 0, :], pTb[:, 1, :]]
                        t1s = [aw.tile([128, TC], F32, tag="t1", name=f"t1{a}",
                                       bufs=4) for a in range(2)]
                        for a in range(2):
                            nc.scalar.activation(t1s[a][:], psLs[a][:], AF.Tanh,
                                                 scale=1.0 / SOFT_CAP)
                        for a in range(2):
                            nc.vector.tensor_tensor(t1s[a][:], t1s[a][:],
                                                    maskT_sb[:, st, :], OP.add)
                        for a in range(2):
                            nc.scalar.activation(pTs[a], t1s[a][:], AF.Exp,
                                                 scale=SOFT_CAP)
                        if pend_pv is not None:
                            pv_step(*pend_pv)
                        pend_pv = (pTb, st)
                    pv_step(*pend_pv)
                    for a, n in enumerate(heads):
                        drow = aw.tile([1, TC], F32, tag="drow", name="drow", bufs=2)
                        nc.vector.reciprocal_approx_fast(drow[:], denb[:, a * TC:(a + 1) * TC])
                        rbden = aw.tile([128, TC], F32, tag="rbden", name="rbden", bufs=2)
                        nc.gpsimd.partition_broadcast(rbden[:], drow[:])
                        for hh in range(2):
                            nc.vector.tensor_tensor(
                                encT_sb[:, n * 2 + hh, :], encs[a][:, hh, :],
                                rbden[:], OP.mult)

            # ---------------- phase C: output projection ----------------
            with tc.tile_pool(name="outp", bufs=3) as outp, \
                 tc.tile_pool(name="ps4", bufs=4, space="PSUM") as ps4:
                for dc in range(4):
                    ow_sb = owp.tile([128, NH * 2, 512], BF16, tag="ow", name="ow_sb")
                    nc.sync.dma_start(
                        ow_sb[:],
                        ow[:, dc * 512:(dc + 1) * 512].rearrange(
                            "(nh p) d -> p nh d", p=128))
                    for tt in range(NTT):
                        psO = ps4.tile([128, 512], F32, tag="psO", name="psO")
                        for nh in range(NH * 2):
                            nc.tensor.matmul(
                                psO[:],
                                encT_sb[:, nh, tt * 128:(tt + 1) * 128],
                                ow_sb[:, nh, :],
                                start=(nh == 0), stop=(nh == NH * 2 - 1))
                        ob = outp.tile([128, 512], F32, tag="ob", name="ob")
                        nc.vector.tensor_copy(ob[:], psO[:])
                        nc.sync.dma_start(
                            out[tt * 128:(tt + 1) * 128, dc * 512:(dc + 1) * 512],
                            ob[:])

    nc.compile()
    return nc


_NC_CACHE = None


def _get_program():
    global _NC_CACHE
    if _NC_CACHE is None:
        _NC_CACHE = build_program()
    return _NC_CACHE


def prepare_inputs(x, q_w, kv_w, o_w, q_scale, k_scale, v_scale, segment_pos,
                   attn_mask):
    """Host-side prep: shard + transpose + fold scales + tables + masks."""
    x = np.asarray(x)
    q_w, kv_w, o_w = np.asarray(q_w), np.asarray(kv_w), np.asarray(o_w)
    q_scale, k_scale, v_scale = (np.asarray(q_scale), np.asarray(k_scale),
                                 np.asarray(v_scale))
    segment_pos = np.asarray(segment_pos)
    attn_mask = np.asarray(attn_mask)
    assert x.shape == (1, T, D)

    qs, ks, vs = 1.0 + q_scale, 1.0 + k_scale, 1.0 + v_scale
    qw_flat = (q_w * qs[None, None, :]).transpose(1, 0, 2).reshape(D, NH * H)
    kwk_flat = (kv_w[0] * ks[None, None, :]).transpose(1, 0, 2).reshape(D, KV * H)
    kwv_flat = (kv_w[1] * vs[None, None, :]).transpose(1, 0, 2).reshape(D, KV * H)
    ow_flat = o_w.reshape(NH * H, D)
    bf = ml_dtypes.bfloat16
    qw_b = np.ascontiguousarray(qw_flat, dtype=bf)
    kwk_b = np.ascontiguousarray(kwk_flat, dtype=bf)
    kwv_b = np.ascontiguousarray(kwv_flat, dtype=bf)
    ow_b = np.ascontiguousarray(ow_flat, dtype=bf)

    inv2q_arr = (qs ** -2.0).reshape(2, HH).T.astype(ml_dtypes.bfloat16)
    inv2k_arr = ((ks ** -2.0) / H).reshape(2, HH).T.astype(ml_dtypes.bfloat16)
    inv2v_arr = (np.tile(vs ** -2.0, KV) / H)[None, :].astype(np.float32)

    pos = segment_pos[0].astype(np.float64)
    freq = ROPE_BASE ** (2.0 * np.arange(HH) / H)
    xt_full = np.ascontiguousarray(x[0].T, dtype=bf)   # [D, T]
    am = attn_mask[0]                                  # [T, T] bool

    t_all = np.arange(T)
    in_maps = []
    for c in range(N_CORES):
        t_lo = c * TC
        xq_c = np.ascontiguousarray(xt_full[:, t_lo:t_lo + TC])

        ang = pos[t_lo:t_lo + TC][None, :] / freq[:, None]   # [HH, TC]
        cosq_c = np.cos(ang).astype(np.float32)
        sinq_c = np.sin(ang).astype(np.float32)

        s_idx = np.arange(t_lo - WINDOW, t_lo + TC)    # [SW]
        valid_s = s_idx >= 0
        sv = s_idx[valid_s]
        t_g = t_all[t_lo:t_lo + TC]
        m = np.zeros((SW, TC), dtype=bool)
        m[valid_s] = am[t_lo:t_lo + TC][:, sv].T
        dwin = t_g[None, :] - s_idx[:, None]
        m &= (dwin >= 0) & (dwin < WINDOW)
        maskT_c = np.where(m, 0.0, -4.0).astype(np.float32).reshape(NST, 128, TC)

        in_maps.append(dict(
            xq=xq_c, qw=qw_b, kwk=kwk_b, kwv=kwv_b, ow=ow_b,
            cosq=cosq_c, sinq=sinq_c, maskT=maskT_c,
            inv2q=inv2q_arr, inv2k=inv2k_arr, inv2v=inv2v_arr,
        ))
    return in_maps


def run(in_maps, trace=False, **kwargs):
    nc = _get_program()
    return run_bass_kernel_spmd(nc, in_maps, core_ids=list(range(N_CORES)),
                                trace=trace, **kwargs)


def kernel(**inputs) -> np.ndarray:
    in_maps = prepare_inputs(**inputs)
    res = run(in_maps)
    out = np.concatenate([res.results[c]["out"] for c in range(N_CORES)], axis=0)
    return out.reshape(1, T, D).astype(np.float32)


if __name__ == "__main__":
    nc = _get_program()
    print("built + compiled OK")


# revision 41
# speedup vs baseline: 1.3603x; 1.0171x over previous
"""Trainium2 Bass kernel for nn_Attention_28802050687686.

GQA sliding-window attention, T=4096, D=2048, 8 Q heads / 4 KV heads,
head_dim 256, window 1024, tanh soft-cap 50, RMSNorm+RoPE on Q/K, RMSNorm on V.

Sharding: sequence-parallel over 8 NeuronCores. Core c owns queries
[512c, 512c+512). Each core computes K/V for its OWN 512 rows only, then an
AllGather (via DRAM) distributes K/V; each core DMAs just its 1536-position
sliding window back into SBUF using partition-id-indexed dynamic offsets
(wrapped mod 8 -- out-of-range chunks land in fully-masked positions).
"""
import sys

sys.path.insert(0, "/opt/trn_rl_repo")

import numpy as np
import ml_dtypes

import concourse.bass as bass
import concourse.tile as tile
from concourse import bacc, mybir
from concourse.bass_utils import run_bass_kernel_spmd

F32 = mybir.dt.float32
BF16 = mybir.dt.bfloat16
AF = mybir.ActivationFunctionType
OP = mybir.AluOpType

# problem constants
T, D, NH, KV, H, HH = 4096, 2048, 8, 4, 256, 128
N_CORES = 8
TC = 512          # queries / own kv rows per core
SW = 1536         # kv window per core
NST = SW // 128   # 12 s-tiles in window
NOT = TC // 128   # 4 own s-tiles
NDT = D // 128    # 16 d-tiles
NTT = TC // 128   # 4 t-tiles
WINDOW = 1024
SOFT_CAP = 50.0
EPS = 1e-6
ROPE_BASE = 10000.0

KCOLS = NH * TC            # 4096 cols of K in the kv-local pack (8 htiles x 512)
VCOLS = NOT * KV * 256     # 4096 cols of V pack
KVCOLS = KCOLS + VCOLS     # 8192


def build_program():
    nc = bacc.Bacc("TRN2", target_bir_lowering=False, debug=False)

    xq = nc.dram_tensor("xq", [D, TC], BF16, kind="ExternalInput").ap()
    qw = nc.dram_tensor("qw", [D, NH * H], BF16, kind="ExternalInput").ap()
    kwk = nc.dram_tensor("kwk", [D, KV * H], BF16, kind="ExternalInput").ap()
    kwv = nc.dram_tensor("kwv", [D, KV * H], BF16, kind="ExternalInput").ap()
    ow = nc.dram_tensor("ow", [NH * H, D], BF16, kind="ExternalInput").ap()
    cosq = nc.dram_tensor("cosq", [HH, TC], F32, kind="ExternalInput").ap()
    sinq = nc.dram_tensor("sinq", [HH, TC], F32, kind="ExternalInput").ap()
    maskT = nc.dram_tensor("maskT", [NST, 128, TC], F32, kind="ExternalInput").ap()
    inv2q = nc.dram_tensor("inv2q", [HH, 2], BF16, kind="ExternalInput").ap()
    inv2k = nc.dram_tensor("inv2k", [HH, 2], BF16, kind="ExternalInput").ap()
    inv2v = nc.dram_tensor("inv2v", [1, KV * H], F32, kind="ExternalInput").ap()
    out = nc.dram_tensor("out", [TC, D], F32, kind="ExternalOutput").ap()

    klocal = nc.dram_tensor("klocal", [128, KCOLS], BF16).ap()
    kgath = nc.dram_tensor("kgath", [N_CORES * 128, KCOLS], BF16,
                           addr_space="Shared").ap()
    vlocal = nc.dram_tensor("vlocal", [128, VCOLS], BF16).ap()
    vgath = nc.dram_tensor("vgath", [N_CORES * 128, VCOLS], BF16,
                           addr_space="Shared").ap()

    with tile.TileContext(nc) as tc:
        with tc.tile_pool(name="persist", bufs=1) as persist, \
             tc.tile_pool(name="work", bufs=2) as work, \
             tc.tile_pool(name="owp", bufs=2) as owp:
            kT_own = persist.tile([128, KV * 2, TC], BF16)    # 8 KB/p
            kT_rem = persist.tile([128, KV * 2, 2 * TC], BF16)  # 16 KB/p
            V_own = persist.tile([128, NOT, KV, 256], BF16)   # 8 KB/p
            V_rem = persist.tile([128, 2 * NOT, KV, 256], BF16)  # 16 KB/p
            qT_g = [persist.tile([128, 4, TC], BF16, name=f"qT{g}")
                    for g in range(KV)]                       # 16 KB/p total
            encT_sb = persist.tile([128, NH * 2, TC], BF16)   # 16 KB/p
            xq_ch = []
            for ch in range(4):
                xc = persist.tile([128, NDT // 4, TC], BF16, name=f"xq{ch}")
                nc.sync.dma_start(
                    xc[:], xq[ch * (D // 4):(ch + 1) * (D // 4), :].rearrange(
                        "(dt p) s -> p dt s", p=128))
                xq_ch.append(xc)

            def xq_sb(dt):
                return xq_ch[dt // 4][:, dt % 4, :]
            cosq_sb = persist.tile([HH, TC], F32)
            nc.sync.dma_start(cosq_sb[:], cosq[:])
            sinq_sb = persist.tile([HH, TC], F32)
            nc.sync.dma_start(sinq_sb[:], sinq[:])
            inv2q_sb = persist.tile([HH, 2], BF16)
            nc.sync.dma_start(inv2q_sb[:], inv2q[:])
            inv2k_sb = persist.tile([HH, 2], BF16)
            nc.sync.dma_start(inv2k_sb[:], inv2k[:])
            inv2v_sb = persist.tile([128, KV * H], F32)       # 4 KB/p
            nc.sync.dma_start(inv2v_sb[:], inv2v.to_broadcast([128, KV * H]))
            epsq1 = persist.tile([1, 1], F32)
            nc.vector.memset(epsq1[:], float(H) * EPS)
            epsk1 = persist.tile([1, 1], F32)
            nc.vector.memset(epsk1[:], EPS)
            eps128 = persist.tile([128, 1], F32)
            nc.vector.memset(eps128[:], EPS)
            ones_f = persist.tile([1, 128], BF16)
            nc.vector.memset(ones_f[:], 1.0)
            ones_b = persist.tile([128, 1], BF16)
            nc.vector.memset(ones_b[:], 1.0)

            def rope_norm_fold(ps_pair, inv2_sb, eps_t, dst0, dst1, bcast):
                """RMSNorm (exact via inv2 weights) + RoPE on an h-pair PSUM
                [128, 2, TC]; writes bf16 to dst0/dst1 [128, TC]."""
                sq0 = work.tile([128, TC], BF16, tag="wsq", name="sq0")
                nc.scalar.activation(sq0[:], ps_pair[:, 0, :], AF.Square)
                sq1 = work.tile([128, TC], BF16, tag="wsq", name="sq1")
                nc.scalar.activation(sq1[:], ps_pair[:, 1, :], AF.Square)
                rps = ps12.tile([1, TC], F32, tag="rowps", name="rps")
                nc.tensor.matmul(rps[:], inv2_sb[:, 0:1], sq0[:],
                                 start=True, stop=False)
                nc.tensor.matmul(rps[:], inv2_sb[:, 1:2], sq1[:],
                                 start=False, stop=True)
                srow = work.tile([1, TC], F32, tag="srow", name="srow")
                nc.scalar.activation(srow[:], rps[:], AF.Sqrt, bias=eps_t[:])
                rrow = work.tile([1, TC], F32, tag="rrow", name="rrow")
                nc.vector.reciprocal_approx_fast(rrow[:], srow[:])
                if bcast == "gpsimd":
                    rb = work.tile([128, TC], F32, tag="rb", name="rb")
                    nc.gpsimd.partition_broadcast(rb[:], rrow[:])
                else:
                    rrow_b = work.tile([1, TC], BF16, tag="rrowb", name="rrow_b")
                    nc.vector.tensor_copy(rrow_b[:], rrow[:])
                    rb = ps12.tile([128, TC], F32, tag="psv", name="rbps")
                    nc.tensor.matmul(rb[:], ones_f[:], rrow_b[:],
                                     start=True, stop=True)
                ta = work.tile([128, TC], F32, tag="wf", name="ta")
                nc.vector.tensor_tensor(ta[:], ps_pair[:, 0, :], cosq_sb[:], OP.mult)
                tb = work.tile([128, TC], F32, tag="wf", name="tb")
                nc.vector.tensor_tensor(tb[:], ps_pair[:, 1, :], sinq_sb[:], OP.mult)
                nc.vector.tensor_tensor(ta[:], ta[:], tb[:], OP.subtract)
                nc.vector.tensor_tensor(dst0, ta[:], rb[:], OP.mult)
                ta2 = work.tile([128, TC], F32, tag="wf", name="ta2")
                nc.vector.tensor_tensor(ta2[:], ps_pair[:, 1, :], cosq_sb[:], OP.mult)
                tb2 = work.tile([128, TC], F32, tag="wf", name="tb2")
                nc.vector.tensor_tensor(tb2[:], ps_pair[:, 0, :], sinq_sb[:], OP.mult)
                nc.vector.tensor_tensor(ta2[:], ta2[:], tb2[:], OP.add)
                nc.vector.tensor_tensor(dst1, ta2[:], rb[:], OP.mult)

            # ---------------- phase A: own-row K/V projections ----------------
            own0 = SW - TC  # own rows start at window col 1024
            with tc.tile_pool(name="wp", bufs=3) as wp, \
                 tc.tile_pool(name="ps12", bufs=2, space="PSUM") as ps12:
                pending = None
                for k in range(KV):
                    wk_sb = wp.tile([128, NDT, H], BF16, tag="wh", name="wk")
                    nc.sync.dma_start(
                        wk_sb[:],
                        kwk[:, k * H:(k + 1) * H].rearrange("(dt p) h -> p dt h", p=128))
                    psp = ps12.tile([128, 2, TC], F32, tag="pspair", name="pspK")
                    for hh in range(2):
                        for dt in range(NDT):
                            nc.tensor.matmul(
                                psp[:, hh, :],
                                wk_sb[:, dt, hh * 128:(hh + 1) * 128],
                                xq_sb(dt),
                                start=(dt == 0), stop=(dt == NDT - 1))
                    if pending is not None:
                        pp, pk = pending
                        rope_norm_fold(pp, inv2k_sb, epsk1,
                                       kT_own[:, pk * 2 + 0, :],
                                       kT_own[:, pk * 2 + 1, :], "gpsimd")
                    pending = (psp, k)
                pp, pk = pending
                rope_norm_fold(pp, inv2k_sb, epsk1,
                               kT_own[:, pk * 2 + 0, :],
                               kT_own[:, pk * 2 + 1, :], "gpsimd")

                nc.gpsimd.dma_start(
                    klocal[:].rearrange("p (a b) -> p a b", a=NH),
                    kT_own[:])
                nc.gpsimd.collective_compute(
                    "AllGather", OP.bypass,
                    replica_groups=[list(range(N_CORES))],
                    ins=[klocal[:]], outs=[kgath[:]],
                )
                pid = nc.gpsimd.partition_id()

                def v_epilogue(psv, k, st):
                    sqv = work.tile([128, H], F32, tag="sqv", name="sqv")
                    nc.scalar.activation(sqv[:], psv[:], AF.Square)
                    sqw = work.tile([128, H], F32, tag="sqw", name="sqw")
                    nc.vector.tensor_tensor(
                        sqw[:], sqv[:], inv2v_sb[:, k * H:(k + 1) * H], OP.mult)
                    rv2 = work.tile([128, 1], F32, tag="rv2", name="rv2")
                    nc.vector.tensor_reduce(rv2[:], sqw[:],
                                            mybir.AxisListType.X, OP.add)
                    srv = work.tile([128, 1], F32, tag="srv", name="srv")
                    nc.scalar.activation(srv[:], rv2[:], AF.Sqrt, bias=eps128[:])
                    rv = work.tile([128, 1], F32, tag="rv", name="rv")
                    nc.vector.reciprocal_approx_fast(rv[:], srv[:])
                    nc.vector.tensor_scalar_mul(
                        V_own[:, st, k, :], psv[:], rv[:])

                pend_v = None
                for k in range(KV):
                    vw_sb = wp.tile([128, NDT, H], BF16, tag="wh", name="vw")
                    nc.sync.dma_start(
                        vw_sb[:],
                        kwv[:, k * H:(k + 1) * H].rearrange("(dt p) h -> p dt h", p=128))
                    for st in range(NOT):
                        psv = ps12.tile([128, H], F32, tag="psv", name="psv")
                        for dt in range(NDT):
                            nc.tensor.matmul(
                                psv[:],
                                xq_sb(dt)[:, st * 128:(st + 1) * 128],
                                vw_sb[:, dt, :],
                                start=(dt == 0), stop=(dt == NDT - 1))
                        if pend_v is not None:
                            v_epilogue(*pend_v)
                        pend_v = (psv, k, st)
                v_epilogue(*pend_v)

                nc.gpsimd.dma_start(
                    vlocal[:].rearrange("p (a k c) -> p a k c", a=NOT, k=KV),
                    V_own[:])
                nc.gpsimd.collective_compute(
                    "AllGather", OP.bypass,
                    replica_groups=[list(range(N_CORES))],
                    ins=[vlocal[:]], outs=[vgath[:]],
                )
                for j in range(2):
                    cj = ((pid + 6 + j) % N_CORES) * 128
                    nc.gpsimd.dma_start(
                        kT_rem[:, :, j * TC:(j + 1) * TC],
                        kgath[bass.ds(cj, 128), :].rearrange(
                            "p (a b) -> p a b", a=NH * 2))
                for j in range(2):
                    cj = ((pid + 6 + j) % N_CORES) * 128
                    nc.gpsimd.dma_start(
                        V_rem[:, NOT * j:NOT * (j + 1), :, :],
                        vgath[bass.ds(cj, 128), :].rearrange(
                            "p (a k c) -> p a k c", a=NOT, k=KV))

                # ------------- phase B1: Q projections (overlap gather) -------
                pend_q = None
                for n in range(NH):
                    wq_sb = wp.tile([128, NDT, H], BF16, tag="wh", name="wq")
                    nc.sync.dma_start(
                        wq_sb[:],
                        qw[:, n * H:(n + 1) * H].rearrange("(dt p) h -> p dt h", p=128))
                    psp = ps12.tile([128, 2, TC], F32, tag="pspair", name="pspQ")
                    for hh in range(2):
                        for dt in range(NDT):
                            nc.tensor.matmul(
                                psp[:, hh, :],
                                wq_sb[:, dt, hh * 128:(hh + 1) * 128],
                                xq_sb(dt),
                                start=(dt == 0), stop=(dt == NDT - 1))
                    if pend_q is not None:
                        pp, pn = pend_q
                        rope_norm_fold(pp, inv2q_sb, epsq1,
                                       qT_g[pn // 2][:, (pn % 2) * 2 + 0, :],
                                       qT_g[pn // 2][:, (pn % 2) * 2 + 1, :], "pe")
                    pend_q = (psp, n)
                pp, pn = pend_q
                rope_norm_fold(pp, inv2q_sb, epsq1,
                               qT_g[pn // 2][:, (pn % 2) * 2 + 0, :],
                               qT_g[pn // 2][:, (pn % 2) * 2 + 1, :], "pe")


            # ---------------- phase B2: attention ----------------
            with tc.tile_pool(name="p3", bufs=1) as p3, \
                 tc.tile_pool(name="aw", bufs=4) as aw, \
                 tc.tile_pool(name="ps3", bufs=2, space="PSUM") as ps3, \
                 tc.tile_pool(name="psenc", bufs=2, space="PSUM") as psenc:
                maskT_sb = p3.tile([128, NST, TC], F32)       # 24 KB/p
                mr = maskT.rearrange("j p t -> p j t")
                nc.sync.dma_start(maskT_sb[:, 0:NST // 2, :], mr[:, 0:NST // 2, :])
                nc.sync.dma_start(maskT_sb[:, NST // 2:, :], mr[:, NST // 2:, :])

                for g in range(KV):
                    heads = (2 * g, 2 * g + 1)
                    encs = [psenc.tile([128, 2, TC], F32, tag="enc",
                                       name=f"enc{a}") for a in range(2)]
                    denb = ps3.tile([1, 2 * TC], F32, tag="den", name="denb",
                                    bufs=1)
                    def pv_step(pTpair, st):
                        # enc.T[h, t] += V.T @ P.T ; den[t] += sum_s P.T
                        # only the in-window query-column range of this s-tile
                        lo, hi = max(0, 128 * (st - 8)), min(TC, 128 * (st + 1))
                        vsl = (V_own[:, st - 8, g, :] if st >= 8
                               else V_rem[:, st, g, :])
                        for hh in range(2):
                            for a in range(2):
                                nc.tensor.matmul(
                                    encs[a][:, hh, lo:hi],
                                    vsl[:, hh * 128:(hh + 1) * 128],
                                    pTpair[:, a, lo:hi],
                                    start=(st == 8), stop=(st == 7))
                        for a in range(2):
                            nc.tensor.matmul(
                                denb[:, a * TC + lo:a * TC + hi], ones_b[:],
                                pTpair[:, a, lo:hi],
                                start=(st == 8), stop=(st == 7))

                    pend_pv = None
                    for st in [8, 9, 10, 11, 0, 1, 2, 3, 4, 5, 6, 7]:
                        if st >= 8:
                            ksl = kT_own[:, :, (st - 8) * 128:(st - 7) * 128]
                        else:
                            ksl = kT_rem[:, :, st * 128:(st + 1) * 128]
                        lo, hi = max(0, 128 * (st - 8)), min(TC, 128 * (st + 1))
                        psLs = [ps3.tile([128, TC], F32, tag="psL",
                                         name=f"psL{a}") for a in range(2)]
                        for hh in range(2):
                            for a, n in enumerate(heads):
                                nc.tensor.matmul(
                                    psLs[a][:, lo:hi],
                                    ksl[:, g * 2 + hh, :],
                                    qT_g[g][:, a * 2 + hh, lo:hi],
                                    start=(hh == 0), stop=(hh == 1))
                        pTb = aw.tile([128, 2, TC], BF16, tag="pT", name="pTb",
                                      bufs=3)
                        pTs = [pTb[:, 0, :], pTb[:, 1, :]]
                        t1s = [aw.tile([128, TC], F32, tag="t1", name=f"t1{a}",
                                       bufs=4) for a in range(2)]
                        for a in range(2):
                            nc.scalar.activation(t1s[a][:], psLs[a][:], AF.Tanh,
                                                 scale=1.0 / SOFT_CAP)
                        for a in range(2):
                            nc.vector.tensor_tensor(t1s[a][:], t1s[a][:],
                                                    maskT_sb[:, st, :], OP.add)
                        for a in range(2):
                            nc.scalar.activation(pTs[a], t1s[a][:], AF.Exp,
                                                 scale=SOFT_CAP)
                        if pend_pv is not None:
                            pv_step(*pend_pv)
                        pend_pv = (pTb, st)
                    pv_step(*pend_pv)
                    for a, n in enumerate(heads):
                        drow = aw.tile([1, TC], F32, tag="drow", name="drow", bufs=2)
                        nc.vector.reciprocal_approx_fast(drow[:], denb[:, a * TC:(a + 1) * TC])
                        rbden = aw.tile([128, TC], F32, tag="rbden", name="rbden", bufs=2)
                        nc.gpsimd.partition_broadcast(rbden[:], drow[:])
                        for hh in range(2):
                            nc.vector.tensor_tensor(
                                encT_sb[:, n * 2 + hh, :], encs[a][:, hh, :],
                                rbden[:], OP.mult)

            # ---------------- phase C: output projection ----------------
            with tc.tile_pool(name="outp", bufs=3) as outp, \
                 tc.tile_pool(name="ps4", bufs=4, space="PSUM") as ps4:
                for dc in range(4):
                    ow_sb = owp.tile([128, NH * 2, 512], BF16, tag="ow", name="ow_sb")
                    nc.sync.dma_start(
                        ow_sb[:],
                        ow[:, dc * 512:(dc + 1) * 512].rearrange(
                            "(nh p) d -> p nh d", p=128))
                    for tt in range(NTT):
                        psO = ps4.tile([128, 512], F32, tag="psO", name="psO")
                        for nh in range(NH * 2):
                            nc.tensor.matmul(
                                psO[:],
                                encT_sb[:, nh, tt * 128:(tt + 1) * 128],
                                ow_sb[:, nh, :],
                                start=(nh == 0), stop=(nh == NH * 2 - 1))
                        ob = outp.tile([128, 512], F32, tag="ob", name="ob")
                        nc.vector.tensor_copy(ob[:], psO[:])
                        nc.sync.dma_start(
                            out[tt * 128:(tt + 1) * 128, dc * 512:(dc + 1) * 512],
                            ob[:])

    nc.compile()
    return nc


_NC_CACHE = None


def _get_program():
    global _NC_CACHE
    if _NC_CACHE is None:
        _NC_CACHE = build_program()
    return _NC_CACHE


def prepare_inputs(x, q_w, kv_w, o_w, q_scale, k_scale, v_scale, segment_pos,
                   attn_mask):
    """Host-side prep: shard + transpose + fold scales + tables + masks."""
    x = np.asarray(x)
    q_w, kv_w, o_w = np.asarray(q_w), np.asarray(kv_w), np.asarray(o_w)
    q_scale, k_scale, v_scale = (np.asarray(q_scale), np.asarray(k_scale),
                                 np.asarray(v_scale))
    segment_pos = np.asarray(segment_pos)
    attn_mask = np.asarray(attn_mask)
    assert x.shape == (1, T, D)

    qs, ks, vs = 1.0 + q_scale, 1.0 + k_scale, 1.0 + v_scale
    qw_flat = (q_w * qs[None, None, :]).transpose(1, 0, 2).reshape(D, NH * H)
    kwk_flat = (kv_w[0] * ks[None, None, :]).transpose(1, 0, 2).reshape(D, KV * H)
    kwv_flat = (kv_w[1] * vs[None, None, :]).transpose(1, 0, 2).reshape(D, KV * H)
    ow_flat = o_w.reshape(NH * H, D)
    bf = ml_dtypes.bfloat16
    qw_b = np.ascontiguousarray(qw_flat, dtype=bf)
    kwk_b = np.ascontiguousarray(kwk_flat, dtype=bf)
    kwv_b = np.ascontiguousarray(kwv_flat, dtype=bf)
    ow_b = np.ascontiguousarray(ow_flat, dtype=bf)

    inv2q_arr = (qs ** -2.0).reshape(2, HH).T.astype(ml_dtypes.bfloat16)
    inv2k_arr = ((ks ** -2.0) / H).reshape(2, HH).T.astype(ml_dtypes.bfloat16)
    inv2v_arr = (np.tile(vs ** -2.0, KV) / H)[None, :].astype(np.float32)

    pos = segment_pos[0].astype(np.float64)
    freq = ROPE_BASE ** (2.0 * np.arange(HH) / H)
    xt_full = np.ascontiguousarray(x[0].T, dtype=bf)   # [D, T]
    am = attn_mask[0]                                  # [T, T] bool

    t_all = np.arange(T)
    in_maps = []
    for c in range(N_CORES):
        t_lo = c * TC
        xq_c = np.ascontiguousarray(xt_full[:, t_lo:t_lo + TC])

        ang = pos[t_lo:t_lo + TC][None, :] / freq[:, None]   # [HH, TC]
        cosq_c = np.cos(ang).astype(np.float32)
        sinq_c = np.sin(ang).astype(np.float32)

        s_idx = np.arange(t_lo - WINDOW, t_lo + TC)    # [SW]
        valid_s = s_idx >= 0
        sv = s_idx[valid_s]
        t_g = t_all[t_lo:t_lo + TC]
        m = np.zeros((SW, TC), dtype=bool)
        m[valid_s] = am[t_lo:t_lo + TC][:, sv].T
        dwin = t_g[None, :] - s_idx[:, None]
        m &= (dwin >= 0) & (dwin < WINDOW)
        maskT_c = np.where(m, 0.0, -4.0).astype(np.float32).reshape(NST, 128, TC)

        in_maps.append(dict(
            xq=xq_c, qw=qw_b, kwk=kwk_b, kwv=kwv_b, ow=ow_b,
            cosq=cosq_c, sinq=sinq_c, maskT=maskT_c,
            inv2q=inv2q_arr, inv2k=inv2k_arr, inv2v=inv2v_arr,
        ))
    return in_maps


def run(in_maps, trace=False, **kwargs):
    nc = _get_program()
    return run_bass_kernel_spmd(nc, in_maps, core_ids=list(range(N_CORES)),
                                trace=trace, **kwargs)


def kernel(**inputs) -> np.ndarray:
    in_maps = prepare_inputs(**inputs)
    res = run(in_maps)
    out = np.concatenate([res.results[c]["out"] for c in range(N_CORES)], axis=0)
    return out.reshape(1, T, D).astype(np.float32)


if __name__ == "__main__":
    nc = _get_program()
    print("built + compiled OK")


# revision 42
# speedup vs baseline: 1.3623x; 1.0015x over previous
"""Trainium2 Bass kernel for nn_Attention_28802050687686.

GQA sliding-window attention, T=4096, D=2048, 8 Q heads / 4 KV heads,
head_dim 256, window 1024, tanh soft-cap 50, RMSNorm+RoPE on Q/K, RMSNorm on V.

Sharding: sequence-parallel over 8 NeuronCores. Core c owns queries
[512c, 512c+512). Each core computes K/V for its OWN 512 rows only, then an
AllGather (via DRAM) distributes K/V; each core DMAs just its 1536-position
sliding window back into SBUF using partition-id-indexed dynamic offsets
(wrapped mod 8 -- out-of-range chunks land in fully-masked positions).
"""
import sys

sys.path.insert(0, "/opt/trn_rl_repo")

import numpy as np
import ml_dtypes

import concourse.bass as bass
import concourse.tile as tile
from concourse import bacc, mybir
from concourse.bass_utils import run_bass_kernel_spmd

F32 = mybir.dt.float32
BF16 = mybir.dt.bfloat16
AF = mybir.ActivationFunctionType
OP = mybir.AluOpType

# problem constants
T, D, NH, KV, H, HH = 4096, 2048, 8, 4, 256, 128
N_CORES = 8
TC = 512          # queries / own kv rows per core
SW = 1536         # kv window per core
NST = SW // 128   # 12 s-tiles in window
NOT = TC // 128   # 4 own s-tiles
NDT = D // 128    # 16 d-tiles
NTT = TC // 128   # 4 t-tiles
WINDOW = 1024
SOFT_CAP = 50.0
EPS = 1e-6
ROPE_BASE = 10000.0

KCOLS = NH * TC            # 4096 cols of K in the kv-local pack (8 htiles x 512)
VCOLS = NOT * KV * 256     # 4096 cols of V pack
KVCOLS = KCOLS + VCOLS     # 8192


def build_program():
    nc = bacc.Bacc("TRN2", target_bir_lowering=False, debug=False)

    xq = nc.dram_tensor("xq", [D, TC], BF16, kind="ExternalInput").ap()
    qw = nc.dram_tensor("qw", [D, NH * H], BF16, kind="ExternalInput").ap()
    kwk = nc.dram_tensor("kwk", [D, KV * H], BF16, kind="ExternalInput").ap()
    kwv = nc.dram_tensor("kwv", [D, KV * H], BF16, kind="ExternalInput").ap()
    ow = nc.dram_tensor("ow", [NH * H, D], BF16, kind="ExternalInput").ap()
    cosq = nc.dram_tensor("cosq", [HH, TC], F32, kind="ExternalInput").ap()
    sinq = nc.dram_tensor("sinq", [HH, TC], F32, kind="ExternalInput").ap()
    maskT = nc.dram_tensor("maskT", [NST, 128, TC], F32, kind="ExternalInput").ap()
    inv2q = nc.dram_tensor("inv2q", [HH, 2], BF16, kind="ExternalInput").ap()
    inv2k = nc.dram_tensor("inv2k", [HH, 2], BF16, kind="ExternalInput").ap()
    inv2v = nc.dram_tensor("inv2v", [1, KV * H], F32, kind="ExternalInput").ap()
    out = nc.dram_tensor("out", [TC, D], F32, kind="ExternalOutput").ap()

    klocal = nc.dram_tensor("klocal", [128, KCOLS], BF16).ap()
    kgath = nc.dram_tensor("kgath", [N_CORES * 128, KCOLS], BF16,
                           addr_space="Shared").ap()
    vlocal = nc.dram_tensor("vlocal", [128, VCOLS], BF16).ap()
    vgath = nc.dram_tensor("vgath", [N_CORES * 128, VCOLS], BF16,
                           addr_space="Shared").ap()

    with tile.TileContext(nc) as tc:
        with tc.tile_pool(name="persist", bufs=1) as persist, \
             tc.tile_pool(name="work", bufs=2) as work, \
             tc.tile_pool(name="owp", bufs=2) as owp:
            kT_own = persist.tile([128, KV * 2, TC], BF16)    # 8 KB/p
            kT_rem = persist.tile([128, KV * 2, 2 * TC], BF16)  # 16 KB/p
            V_own = persist.tile([128, NOT, KV, 256], BF16)   # 8 KB/p
            V_rem = persist.tile([128, 2 * NOT, KV, 256], BF16)  # 16 KB/p
            qT_g = [persist.tile([128, 4, TC], BF16, name=f"qT{g}")
                    for g in range(KV)]                       # 16 KB/p total
            encT_sb = persist.tile([128, NH * 2, TC], BF16)   # 16 KB/p
            xq_ch = []
            for ch in range(4):
                xc = persist.tile([128, NDT // 4, TC], BF16, name=f"xq{ch}")
                nc.sync.dma_start(
                    xc[:], xq[ch * (D // 4):(ch + 1) * (D // 4), :].rearrange(
                        "(dt p) s -> p dt s", p=128))
                xq_ch.append(xc)

            def xq_sb(dt):
                return xq_ch[dt // 4][:, dt % 4, :]
            cosq_sb = persist.tile([HH, TC], F32)
            nc.sync.dma_start(cosq_sb[:], cosq[:])
            sinq_sb = persist.tile([HH, TC], F32)
            nc.sync.dma_start(sinq_sb[:], sinq[:])
            inv2q_sb = persist.tile([HH, 2], BF16)
            nc.sync.dma_start(inv2q_sb[:], inv2q[:])
            inv2k_sb = persist.tile([HH, 2], BF16)
            nc.sync.dma_start(inv2k_sb[:], inv2k[:])
            inv2v_sb = persist.tile([128, KV * H], F32)       # 4 KB/p
            nc.sync.dma_start(inv2v_sb[:], inv2v.to_broadcast([128, KV * H]))
            epsq1 = persist.tile([1, 1], F32)
            nc.vector.memset(epsq1[:], float(H) * EPS)
            epsk1 = persist.tile([1, 1], F32)
            nc.vector.memset(epsk1[:], EPS)
            eps128 = persist.tile([128, 1], F32)
            nc.vector.memset(eps128[:], EPS)
            ones_f = persist.tile([1, 128], BF16)
            nc.vector.memset(ones_f[:], 1.0)
            ones_b = persist.tile([128, 1], BF16)
            nc.vector.memset(ones_b[:], 1.0)

            def rope_norm_fold(ps_pair, inv2_sb, eps_t, dst0, dst1, bcast):
                """RMSNorm (exact via inv2 weights) + RoPE on an h-pair PSUM
                [128, 2, TC]; writes bf16 to dst0/dst1 [128, TC]."""
                sq0 = work.tile([128, TC], BF16, tag="wsq", name="sq0")
                nc.scalar.activation(sq0[:], ps_pair[:, 0, :], AF.Square)
                sq1 = work.tile([128, TC], BF16, tag="wsq", name="sq1")
                nc.scalar.activation(sq1[:], ps_pair[:, 1, :], AF.Square)
                rps = ps12.tile([1, TC], F32, tag="rowps", name="rps")
                nc.tensor.matmul(rps[:], inv2_sb[:, 0:1], sq0[:],
                                 start=True, stop=False)
                nc.tensor.matmul(rps[:], inv2_sb[:, 1:2], sq1[:],
                                 start=False, stop=True)
                srow = work.tile([1, TC], F32, tag="srow", name="srow")
                nc.scalar.activation(srow[:], rps[:], AF.Sqrt, bias=eps_t[:])
                rrow = work.tile([1, TC], F32, tag="rrow", name="rrow")
                nc.vector.reciprocal_approx_fast(rrow[:], srow[:])
                if bcast == "gpsimd":
                    rb = work.tile([128, TC], F32, tag="rb", name="rb")
                    nc.gpsimd.partition_broadcast(rb[:], rrow[:])
                else:
                    rrow_b = work.tile([1, TC], BF16, tag="rrowb", name="rrow_b")
                    nc.vector.tensor_copy(rrow_b[:], rrow[:])
                    rb = ps12.tile([128, TC], F32, tag="psv", name="rbps")
                    nc.tensor.matmul(rb[:], ones_f[:], rrow_b[:],
                                     start=True, stop=True)
                ta = work.tile([128, TC], F32, tag="wf", name="ta")
                nc.vector.tensor_tensor(ta[:], ps_pair[:, 0, :], cosq_sb[:], OP.mult)
                tb = work.tile([128, TC], F32, tag="wf", name="tb")
                nc.vector.tensor_tensor(tb[:], ps_pair[:, 1, :], sinq_sb[:], OP.mult)
                nc.vector.tensor_tensor(ta[:], ta[:], tb[:], OP.subtract)
                nc.vector.tensor_tensor(dst0, ta[:], rb[:], OP.mult)
                ta2 = work.tile([128, TC], F32, tag="wf", name="ta2")
                nc.vector.tensor_tensor(ta2[:], ps_pair[:, 1, :], cosq_sb[:], OP.mult)
                tb2 = work.tile([128, TC], F32, tag="wf", name="tb2")
                nc.vector.tensor_tensor(tb2[:], ps_pair[:, 0, :], sinq_sb[:], OP.mult)
                nc.vector.tensor_tensor(ta2[:], ta2[:], tb2[:], OP.add)
                nc.vector.tensor_tensor(dst1, ta2[:], rb[:], OP.mult)

            # ---------------- phase A: own-row K/V projections ----------------
            own0 = SW - TC  # own rows start at window col 1024
            with tc.tile_pool(name="wp", bufs=3) as wp, \
                 tc.tile_pool(name="ps12", bufs=2, space="PSUM") as ps12:
                pending = None
                for k in range(KV):
                    wk_sb = wp.tile([128, NDT, H], BF16, tag="wh", name="wk")
                    nc.sync.dma_start(
                        wk_sb[:],
                        kwk[:, k * H:(k + 1) * H].rearrange("(dt p) h -> p dt h", p=128))
                    psp = ps12.tile([128, 2, TC], F32, tag="pspair", name="pspK")
                    for hh in range(2):
                        for dt in range(NDT):
                            nc.tensor.matmul(
                                psp[:, hh, :],
                                wk_sb[:, dt, hh * 128:(hh + 1) * 128],
                                xq_sb(dt),
                                start=(dt == 0), stop=(dt == NDT - 1))
                    if pending is not None:
                        pp, pk = pending
                        rope_norm_fold(pp, inv2k_sb, epsk1,
                                       kT_own[:, pk * 2 + 0, :],
                                       kT_own[:, pk * 2 + 1, :], "gpsimd")
                    pending = (psp, k)
                pp, pk = pending
                rope_norm_fold(pp, inv2k_sb, epsk1,
                               kT_own[:, pk * 2 + 0, :],
                               kT_own[:, pk * 2 + 1, :], "gpsimd")

                nc.gpsimd.dma_start(
                    klocal[:].rearrange("p (a b) -> p a b", a=NH),
                    kT_own[:])
                nc.gpsimd.collective_compute(
                    "AllGather", OP.bypass,
                    replica_groups=[list(range(N_CORES))],
                    ins=[klocal[:]], outs=[kgath[:]],
                )
                pid = nc.gpsimd.partition_id()

                def v_epilogue(psv, k, st):
                    sqv = work.tile([128, H], F32, tag="sqv", name="sqv")
                    nc.scalar.activation(sqv[:], psv[:], AF.Square)
                    sqw = work.tile([128, H], F32, tag="sqw", name="sqw")
                    nc.vector.tensor_tensor(
                        sqw[:], sqv[:], inv2v_sb[:, k * H:(k + 1) * H], OP.mult)
                    rv2 = work.tile([128, 1], F32, tag="rv2", name="rv2")
                    nc.vector.tensor_reduce(rv2[:], sqw[:],
                                            mybir.AxisListType.X, OP.add)
                    srv = work.tile([128, 1], F32, tag="srv", name="srv")
                    nc.scalar.activation(srv[:], rv2[:], AF.Sqrt, bias=eps128[:])
                    rv = work.tile([128, 1], F32, tag="rv", name="rv")
                    nc.vector.reciprocal_approx_fast(rv[:], srv[:])
                    nc.vector.tensor_scalar_mul(
                        V_own[:, st, k, :], psv[:], rv[:])

                pend_v = None
                for k in range(KV):
                    vw_sb = wp.tile([128, NDT, H], BF16, tag="wh", name="vw")
                    nc.sync.dma_start(
                        vw_sb[:],
                        kwv[:, k * H:(k + 1) * H].rearrange("(dt p) h -> p dt h", p=128))
                    for st in range(NOT):
                        psv = ps12.tile([128, H], F32, tag="psv", name="psv")
                        for dt in range(NDT):
                            nc.tensor.matmul(
                                psv[:],
                                xq_sb(dt)[:, st * 128:(st + 1) * 128],
                                vw_sb[:, dt, :],
                                start=(dt == 0), stop=(dt == NDT - 1))
                        if pend_v is not None:
                            v_epilogue(*pend_v)
                        pend_v = (psv, k, st)
                v_epilogue(*pend_v)

                nc.gpsimd.dma_start(
                    vlocal[:].rearrange("p (a k c) -> p a k c", a=NOT, k=KV),
                    V_own[:])
                nc.gpsimd.collective_compute(
                    "AllGather", OP.bypass,
                    replica_groups=[list(range(N_CORES))],
                    ins=[vlocal[:]], outs=[vgath[:]],
                )
                for j in range(2):
                    cj = ((pid + 6 + j) % N_CORES) * 128
                    nc.gpsimd.dma_start(
                        kT_rem[:, :, j * TC:(j + 1) * TC],
                        kgath[bass.ds(cj, 128), :].rearrange(
                            "p (a b) -> p a b", a=NH * 2))
                for j in range(2):
                    cj = ((pid + 6 + j) % N_CORES) * 128
                    nc.gpsimd.dma_start(
                        V_rem[:, NOT * j:NOT * (j + 1), :, :],
                        vgath[bass.ds(cj, 128), :].rearrange(
                            "p (a k c) -> p a k c", a=NOT, k=KV))

                # ------------- phase B1: Q projections (overlap gather) -------
                pend_q = None
                for n in range(NH):
                    wq_sb = wp.tile([128, NDT, H], BF16, tag="wh", name="wq")
                    nc.sync.dma_start(
                        wq_sb[:],
                        qw[:, n * H:(n + 1) * H].rearrange("(dt p) h -> p dt h", p=128))
                    psp = ps12.tile([128, 2, TC], F32, tag="pspair", name="pspQ")
                    for hh in range(2):
                        for dt in range(NDT):
                            nc.tensor.matmul(
                                psp[:, hh, :],
                                wq_sb[:, dt, hh * 128:(hh + 1) * 128],
                                xq_sb(dt),
                                start=(dt == 0), stop=(dt == NDT - 1))
                    if pend_q is not None:
                        pp, pn = pend_q
                        rope_norm_fold(pp, inv2q_sb, epsq1,
                                       qT_g[pn // 2][:, (pn % 2) * 2 + 0, :],
                                       qT_g[pn // 2][:, (pn % 2) * 2 + 1, :], "pe")
                    pend_q = (psp, n)
                pp, pn = pend_q
                rope_norm_fold(pp, inv2q_sb, epsq1,
                               qT_g[pn // 2][:, (pn % 2) * 2 + 0, :],
                               qT_g[pn // 2][:, (pn % 2) * 2 + 1, :], "pe")


            # ---------------- phase B2: attention ----------------
            with tc.tile_pool(name="p3", bufs=1) as p3, \
                 tc.tile_pool(name="aw", bufs=4) as aw, \
                 tc.tile_pool(name="ps3", bufs=2, space="PSUM") as ps3, \
                 tc.tile_pool(name="psenc", bufs=2, space="PSUM") as psenc:
                maskT_sb = p3.tile([128, NST, TC], F32)       # 24 KB/p
                mr = maskT.rearrange("j p t -> p j t")
                nc.sync.dma_start(maskT_sb[:, 0:NST // 2, :], mr[:, 0:NST // 2, :])
                nc.sync.dma_start(maskT_sb[:, NST // 2:, :], mr[:, NST // 2:, :])

                for g in range(KV):
                    heads = (2 * g, 2 * g + 1)
                    encs = [psenc.tile([128, 2, TC], F32, tag="enc",
                                       name=f"enc{a}") for a in range(2)]
                    denb = ps3.tile([1, 2 * TC], F32, tag="den", name="denb",
                                    bufs=1)
                    def pv_step(pTpair, st):
                        # enc.T[h, t] += V.T @ P.T ; den[t] += sum_s P.T
                        # only the in-window query-column range of this s-tile
                        lo, hi = max(0, 128 * (st - 8)), min(TC, 128 * (st + 1))
                        vsl = (V_own[:, st - 8, g, :] if st >= 8
                               else V_rem[:, st, g, :])
                        for hh in range(2):
                            for a in range(2):
                                nc.tensor.matmul(
                                    encs[a][:, hh, lo:hi],
                                    vsl[:, hh * 128:(hh + 1) * 128],
                                    pTpair[:, a, lo:hi],
                                    start=(st == 8), stop=(st == 7))
                        for a in range(2):
                            nc.tensor.matmul(
                                denb[:, a * TC + lo:a * TC + hi], ones_b[:],
                                pTpair[:, a, lo:hi],
                                start=(st == 8), stop=(st == 7))

                    pend_pv = None
                    for st in [8, 9, 10, 11, 0, 1, 2, 3, 4, 5, 6, 7]:
                        if st >= 8:
                            ksl = kT_own[:, :, (st - 8) * 128:(st - 7) * 128]
                        else:
                            ksl = kT_rem[:, :, st * 128:(st + 1) * 128]
                        lo, hi = max(0, 128 * (st - 8)), min(TC, 128 * (st + 1))
                        psLs = [ps3.tile([128, TC], F32, tag="psL",
                                         name=f"psL{a}") for a in range(2)]
                        for hh in range(2):
                            for a, n in enumerate(heads):
                                nc.tensor.matmul(
                                    psLs[a][:, lo:hi],
                                    ksl[:, g * 2 + hh, :],
                                    qT_g[g][:, a * 2 + hh, lo:hi],
                                    start=(hh == 0), stop=(hh == 1))
                        pTb = aw.tile([128, 2, TC], BF16, tag="pT", name="pTb",
                                      bufs=3)
                        t1s = [aw.tile([128, TC], F32, tag="t1", name=f"t1{a}",
                                       bufs=4) for a in range(2)]
                        for a in range(2):
                            nc.scalar.activation(t1s[a][:, lo:hi],
                                                 psLs[a][:, lo:hi], AF.Tanh,
                                                 scale=1.0 / SOFT_CAP)
                        for a in range(2):
                            nc.vector.tensor_tensor(t1s[a][:, lo:hi],
                                                    t1s[a][:, lo:hi],
                                                    maskT_sb[:, st, lo:hi],
                                                    OP.add)
                        for a in range(2):
                            nc.scalar.activation(pTb[:, a, lo:hi],
                                                 t1s[a][:, lo:hi], AF.Exp,
                                                 scale=SOFT_CAP)
                        if pend_pv is not None:
                            pv_step(*pend_pv)
                        pend_pv = (pTb, st)
                    pv_step(*pend_pv)
                    for a, n in enumerate(heads):
                        drow = aw.tile([1, TC], F32, tag="drow", name="drow", bufs=2)
                        nc.vector.reciprocal_approx_fast(drow[:], denb[:, a * TC:(a + 1) * TC])
                        rbden = aw.tile([128, TC], F32, tag="rbden", name="rbden", bufs=2)
                        nc.gpsimd.partition_broadcast(rbden[:], drow[:])
                        for hh in range(2):
                            nc.vector.tensor_tensor(
                                encT_sb[:, n * 2 + hh, :], encs[a][:, hh, :],
                                rbden[:], OP.mult)

            # ---------------- phase C: output projection ----------------
            with tc.tile_pool(name="outp", bufs=3) as outp, \
                 tc.tile_pool(name="ps4", bufs=4, space="PSUM") as ps4:
                for dc in range(4):
                    ow_sb = owp.tile([128, NH * 2, 512], BF16, tag="ow", name="ow_sb")
                    nc.sync.dma_start(
                        ow_sb[:],
                        ow[:, dc * 512:(dc + 1) * 512].rearrange(
                            "(nh p) d -> p nh d", p=128))
                    for tt in range(NTT):
                        psO = ps4.tile([128, 512], F32, tag="psO", name="psO")
                        for nh in range(NH * 2):
                            nc.tensor.matmul(
                                psO[:],
                                encT_sb[:, nh, tt * 128:(tt + 1) * 128],
                                ow_sb[:, nh, :],
                                start=(nh == 0), stop=(nh == NH * 2 - 1))
                        ob = outp.tile([128, 512], F32, tag="ob", name="ob")
                        nc.vector.tensor_copy(ob[:], psO[:])
                        nc.sync.dma_start(
                            out[tt * 128:(tt + 1) * 128, dc * 512:(dc + 1) * 512],
                            ob[:])

    nc.compile()
    return nc


_NC_CACHE = None


def _get_program():
    global _NC_CACHE
    if _NC_CACHE is None:
        _NC_CACHE = build_program()
    return _NC_CACHE


def prepare_inputs(x, q_w, kv_w, o_w, q_scale, k_scale, v_scale, segment_pos,
                   attn_mask):
    """Host-side prep: shard + transpose + fold scales + tables + masks."""
    x = np.asarray(x)
    q_w, kv_w, o_w = np.asarray(q_w), np.asarray(kv_w), np.asarray(o_w)
    q_scale, k_scale, v_scale = (np.asarray(q_scale), np.asarray(k_scale),
                                 np.asarray(v_scale))
    segment_pos = np.asarray(segment_pos)
    attn_mask = np.asarray(attn_mask)
    assert x.shape == (1, T, D)

    qs, ks, vs = 1.0 + q_scale, 1.0 + k_scale, 1.0 + v_scale
    qw_flat = (q_w * qs[None, None, :]).transpose(1, 0, 2).reshape(D, NH * H)
    kwk_flat = (kv_w[0] * ks[None, None, :]).transpose(1, 0, 2).reshape(D, KV * H)
    kwv_flat = (kv_w[1] * vs[None, None, :]).transpose(1, 0, 2).reshape(D, KV * H)
    ow_flat = o_w.reshape(NH * H, D)
    bf = ml_dtypes.bfloat16
    qw_b = np.ascontiguousarray(qw_flat, dtype=bf)
    kwk_b = np.ascontiguousarray(kwk_flat, dtype=bf)
    kwv_b = np.ascontiguousarray(kwv_flat, dtype=bf)
    ow_b = np.ascontiguousarray(ow_flat, dtype=bf)

    inv2q_arr = (qs ** -2.0).reshape(2, HH).T.astype(ml_dtypes.bfloat16)
    inv2k_arr = ((ks ** -2.0) / H).reshape(2, HH).T.astype(ml_dtypes.bfloat16)
    inv2v_arr = (np.tile(vs ** -2.0, KV) / H)[None, :].astype(np.float32)

    pos = segment_pos[0].astype(np.float64)
    freq = ROPE_BASE ** (2.0 * np.arange(HH) / H)
    xt_full = np.ascontiguousarray(x[0].T, dtype=bf)   # [D, T]
    am = attn_mask[0]                                  # [T, T] bool

    t_all = np.arange(T)
    in_maps = []
    for c in range(N_CORES):
        t_lo = c * TC
        xq_c = np.ascontiguousarray(xt_full[:, t_lo:t_lo + TC])

        ang = pos[t_lo:t_lo + TC][None, :] / freq[:, None]   # [HH, TC]
        cosq_c = np.cos(ang).astype(np.float32)
        sinq_c = np.sin(ang).astype(np.float32)

        s_idx = np.arange(t_lo - WINDOW, t_lo + TC)    # [SW]
        valid_s = s_idx >= 0
        sv = s_idx[valid_s]
        t_g = t_all[t_lo:t_lo + TC]
        m = np.zeros((SW, TC), dtype=bool)
        m[valid_s] = am[t_lo:t_lo + TC][:, sv].T
        dwin = t_g[None, :] - s_idx[:, None]
        m &= (dwin >= 0) & (dwin < WINDOW)
        maskT_c = np.where(m, 0.0, -4.0).astype(np.float32).reshape(NST, 128, TC)

        in_maps.append(dict(
            xq=xq_c, qw=qw_b, kwk=kwk_b, kwv=kwv_b, ow=ow_b,
            cosq=cosq_c, sinq=sinq_c, maskT=maskT_c,
            inv2q=inv2q_arr, inv2k=inv2k_arr, inv2v=inv2v_arr,
        ))
    return in_maps


def run(in_maps, trace=False, **kwargs):
    nc = _get_program()
    return run_bass_kernel_spmd(nc, in_maps, core_ids=list(range(N_CORES)),
                                trace=trace, **kwargs)


def kernel(**inputs) -> np.ndarray:
    in_maps = prepare_inputs(**inputs)
    res = run(in_maps)
    out = np.concatenate([res.results[c]["out"] for c in range(N_CORES)], axis=0)
    return out.reshape(1, T, D).astype(np.float32)


if __name__ == "__main__":
    nc = _get_program()
    print("built + compiled OK")


# revision 43
# speedup vs baseline: 1.4116x; 1.0362x over previous
"""Trainium2 Bass kernel for nn_Attention_28802050687686.

GQA sliding-window attention, T=4096, D=2048, 8 Q heads / 4 KV heads,
head_dim 256, window 1024, tanh soft-cap 50, RMSNorm+RoPE on Q/K, RMSNorm on V.

Sharding: sequence-parallel over 8 NeuronCores. Core c owns queries
[512c, 512c+512). Each core computes K/V for its OWN 512 rows only, then an
AllGather (via DRAM) distributes K/V; each core DMAs just its 1536-position
sliding window back into SBUF using partition-id-indexed dynamic offsets
(wrapped mod 8 -- out-of-range chunks land in fully-masked positions).
"""
import sys

sys.path.insert(0, "/opt/trn_rl_repo")

import numpy as np
import ml_dtypes

import concourse.bass as bass
import concourse.tile as tile
from concourse import bacc, mybir
from concourse.bass_utils import run_bass_kernel_spmd

F32 = mybir.dt.float32
BF16 = mybir.dt.bfloat16
AF = mybir.ActivationFunctionType
OP = mybir.AluOpType

# problem constants
T, D, NH, KV, H, HH = 4096, 2048, 8, 4, 256, 128
N_CORES = 8
TC = 512          # queries / own kv rows per core
SW = 1536         # kv window per core
NST = SW // 128   # 12 s-tiles in window
NOT = TC // 128   # 4 own s-tiles
NDT = D // 128    # 16 d-tiles
NTT = TC // 128   # 4 t-tiles
WINDOW = 1024
SOFT_CAP = 50.0
EPS = 1e-6
ROPE_BASE = 10000.0

KCOLS = NH * TC            # 4096 cols of K in the kv-local pack (8 htiles x 512)
VCOLS = NOT * KV * 256     # 4096 cols of V pack
KVCOLS = KCOLS + VCOLS     # 8192


def build_program():
    nc = bacc.Bacc("TRN2", target_bir_lowering=False, debug=False)

    xq = nc.dram_tensor("xq", [D, TC], BF16, kind="ExternalInput").ap()
    qw = nc.dram_tensor("qw", [D, NH * H], BF16, kind="ExternalInput").ap()
    kwk = nc.dram_tensor("kwk", [D, KV * H], BF16, kind="ExternalInput").ap()
    kwv = nc.dram_tensor("kwv", [D, KV * H], BF16, kind="ExternalInput").ap()
    ow = nc.dram_tensor("ow", [NH * H, D], BF16, kind="ExternalInput").ap()
    cosq = nc.dram_tensor("cosq", [HH, TC], F32, kind="ExternalInput").ap()
    sinq = nc.dram_tensor("sinq", [HH, TC], F32, kind="ExternalInput").ap()
    maskT = nc.dram_tensor("maskT", [NST, 128, TC], F32, kind="ExternalInput").ap()
    inv2q = nc.dram_tensor("inv2q", [HH, 2], BF16, kind="ExternalInput").ap()
    inv2k = nc.dram_tensor("inv2k", [HH, 2], BF16, kind="ExternalInput").ap()
    inv2v = nc.dram_tensor("inv2v", [1, KV * H], F32, kind="ExternalInput").ap()
    out = nc.dram_tensor("out", [TC, D], F32, kind="ExternalOutput").ap()

    kvlocal = nc.dram_tensor("kvlocal", [128, KVCOLS], BF16).ap()
    kvgath = nc.dram_tensor("kvgath", [N_CORES * 128, KVCOLS], BF16,
                            addr_space="Shared").ap()

    with tile.TileContext(nc) as tc:
        with tc.tile_pool(name="persist", bufs=1) as persist, \
             tc.tile_pool(name="work", bufs=2) as work, \
             tc.tile_pool(name="owp", bufs=2) as owp:
            kT_own = persist.tile([128, KV * 2, TC], BF16)    # 8 KB/p
            kT_rem = persist.tile([128, KV * 2, 2 * TC], BF16)  # 16 KB/p
            V_own = persist.tile([128, NOT, KV, 256], BF16)   # 8 KB/p
            V_rem = persist.tile([128, 2 * NOT, KV, 256], BF16)  # 16 KB/p
            qT_g = [persist.tile([128, 4, TC], BF16, name=f"qT{g}")
                    for g in range(KV)]                       # 16 KB/p total
            encT_sb = persist.tile([128, NH * 2, TC], BF16)   # 16 KB/p
            xq_ch = []
            for ch in range(4):
                xc = persist.tile([128, NDT // 4, TC], BF16, name=f"xq{ch}")
                nc.sync.dma_start(
                    xc[:], xq[ch * (D // 4):(ch + 1) * (D // 4), :].rearrange(
                        "(dt p) s -> p dt s", p=128))
                xq_ch.append(xc)

            def xq_sb(dt):
                return xq_ch[dt // 4][:, dt % 4, :]
            cosq_sb = persist.tile([HH, TC], F32)
            nc.sync.dma_start(cosq_sb[:], cosq[:])
            sinq_sb = persist.tile([HH, TC], F32)
            nc.sync.dma_start(sinq_sb[:], sinq[:])
            inv2q_sb = persist.tile([HH, 2], BF16)
            nc.sync.dma_start(inv2q_sb[:], inv2q[:])
            inv2k_sb = persist.tile([HH, 2], BF16)
            nc.sync.dma_start(inv2k_sb[:], inv2k[:])
            inv2v_sb = persist.tile([128, KV * H], F32)       # 4 KB/p
            nc.sync.dma_start(inv2v_sb[:], inv2v.to_broadcast([128, KV * H]))
            epsq1 = persist.tile([1, 1], F32)
            nc.vector.memset(epsq1[:], float(H) * EPS)
            epsk1 = persist.tile([1, 1], F32)
            nc.vector.memset(epsk1[:], EPS)
            eps128 = persist.tile([128, 1], F32)
            nc.vector.memset(eps128[:], EPS)
            ones_f = persist.tile([1, 128], BF16)
            nc.vector.memset(ones_f[:], 1.0)
            ones_b = persist.tile([128, 1], BF16)
            nc.vector.memset(ones_b[:], 1.0)

            def rope_norm_fold(ps_pair, inv2_sb, eps_t, dst0, dst1, bcast):
                """RMSNorm (exact via inv2 weights) + RoPE on an h-pair PSUM
                [128, 2, TC]; writes bf16 to dst0/dst1 [128, TC]."""
                sq0 = work.tile([128, TC], BF16, tag="wsq", name="sq0")
                nc.scalar.activation(sq0[:], ps_pair[:, 0, :], AF.Square)
                sq1 = work.tile([128, TC], BF16, tag="wsq", name="sq1")
                nc.scalar.activation(sq1[:], ps_pair[:, 1, :], AF.Square)
                rps = ps12.tile([1, TC], F32, tag="rowps", name="rps")
                nc.tensor.matmul(rps[:], inv2_sb[:, 0:1], sq0[:],
                                 start=True, stop=False)
                nc.tensor.matmul(rps[:], inv2_sb[:, 1:2], sq1[:],
                                 start=False, stop=True)
                srow = work.tile([1, TC], F32, tag="srow", name="srow")
                nc.scalar.activation(srow[:], rps[:], AF.Sqrt, bias=eps_t[:])
                rrow = work.tile([1, TC], F32, tag="rrow", name="rrow")
                nc.vector.reciprocal_approx_fast(rrow[:], srow[:])
                if bcast == "gpsimd":
                    rb = work.tile([128, TC], F32, tag="rb", name="rb")
                    nc.gpsimd.partition_broadcast(rb[:], rrow[:])
                else:
                    rrow_b = work.tile([1, TC], BF16, tag="rrowb", name="rrow_b")
                    nc.vector.tensor_copy(rrow_b[:], rrow[:])
                    rb = ps12.tile([128, TC], F32, tag="psv", name="rbps")
                    nc.tensor.matmul(rb[:], ones_f[:], rrow_b[:],
                                     start=True, stop=True)
                ta = work.tile([128, TC], F32, tag="wf", name="ta")
                nc.vector.tensor_tensor(ta[:], ps_pair[:, 0, :], cosq_sb[:], OP.mult)
                tb = work.tile([128, TC], F32, tag="wf", name="tb")
                nc.vector.tensor_tensor(tb[:], ps_pair[:, 1, :], sinq_sb[:], OP.mult)
                nc.vector.tensor_tensor(ta[:], ta[:], tb[:], OP.subtract)
                nc.vector.tensor_tensor(dst0, ta[:], rb[:], OP.mult)
                ta2 = work.tile([128, TC], F32, tag="wf", name="ta2")
                nc.vector.tensor_tensor(ta2[:], ps_pair[:, 1, :], cosq_sb[:], OP.mult)
                tb2 = work.tile([128, TC], F32, tag="wf", name="tb2")
                nc.vector.tensor_tensor(tb2[:], ps_pair[:, 0, :], sinq_sb[:], OP.mult)
                nc.vector.tensor_tensor(ta2[:], ta2[:], tb2[:], OP.add)
                nc.vector.tensor_tensor(dst1, ta2[:], rb[:], OP.mult)

            # ---------------- phase A: own-row K/V projections ----------------
            own0 = SW - TC  # own rows start at window col 1024
            with tc.tile_pool(name="wp", bufs=3) as wp, \
                 tc.tile_pool(name="ps12", bufs=2, space="PSUM") as ps12:
                pending = None
                for k in range(KV):
                    wk_sb = wp.tile([128, NDT, H], BF16, tag="wh", name="wk")
                    nc.sync.dma_start(
                        wk_sb[:],
                        kwk[:, k * H:(k + 1) * H].rearrange("(dt p) h -> p dt h", p=128))
                    psp = ps12.tile([128, 2, TC], F32, tag="pspair", name="pspK")
                    for hh in range(2):
                        for dt in range(NDT):
                            nc.tensor.matmul(
                                psp[:, hh, :],
                                wk_sb[:, dt, hh * 128:(hh + 1) * 128],
                                xq_sb(dt),
                                start=(dt == 0), stop=(dt == NDT - 1))
                    if pending is not None:
                        pp, pk = pending
                        rope_norm_fold(pp, inv2k_sb, epsk1,
                                       kT_own[:, pk * 2 + 0, :],
                                       kT_own[:, pk * 2 + 1, :], "gpsimd")
                    pending = (psp, k)
                pp, pk = pending
                rope_norm_fold(pp, inv2k_sb, epsk1,
                               kT_own[:, pk * 2 + 0, :],
                               kT_own[:, pk * 2 + 1, :], "gpsimd")

                nc.gpsimd.dma_start(
                    kvlocal[:, 0:KCOLS].rearrange("p (a b) -> p a b", a=NH),
                    kT_own[:])
                pid = nc.gpsimd.partition_id()

                def v_epilogue(psv, k, st):
                    sqv = work.tile([128, H], F32, tag="sqv", name="sqv")
                    nc.scalar.activation(sqv[:], psv[:], AF.Square)
                    sqw = work.tile([128, H], F32, tag="sqw", name="sqw")
                    nc.vector.tensor_tensor(
                        sqw[:], sqv[:], inv2v_sb[:, k * H:(k + 1) * H], OP.mult)
                    rv2 = work.tile([128, 1], F32, tag="rv2", name="rv2")
                    nc.vector.tensor_reduce(rv2[:], sqw[:],
                                            mybir.AxisListType.X, OP.add)
                    srv = work.tile([128, 1], F32, tag="srv", name="srv")
                    nc.scalar.activation(srv[:], rv2[:], AF.Sqrt, bias=eps128[:])
                    rv = work.tile([128, 1], F32, tag="rv", name="rv")
                    nc.vector.reciprocal_approx_fast(rv[:], srv[:])
                    nc.vector.tensor_scalar_mul(
                        V_own[:, st, k, :], psv[:], rv[:])

                pend_v = None
                for k in range(KV):
                    vw_sb = wp.tile([128, NDT, H], BF16, tag="wh", name="vw")
                    nc.sync.dma_start(
                        vw_sb[:],
                        kwv[:, k * H:(k + 1) * H].rearrange("(dt p) h -> p dt h", p=128))
                    for st in range(NOT):
                        psv = ps12.tile([128, H], F32, tag="psv", name="psv")
                        for dt in range(NDT):
                            nc.tensor.matmul(
                                psv[:],
                                xq_sb(dt)[:, st * 128:(st + 1) * 128],
                                vw_sb[:, dt, :],
                                start=(dt == 0), stop=(dt == NDT - 1))
                        if pend_v is not None:
                            v_epilogue(*pend_v)
                        pend_v = (psv, k, st)
                v_epilogue(*pend_v)

                nc.gpsimd.dma_start(
                    kvlocal[:, KCOLS:KVCOLS].rearrange(
                        "p (a k c) -> p a k c", a=NOT, k=KV),
                    V_own[:])
                nc.gpsimd.collective_compute(
                    "AllGather", OP.bypass,
                    replica_groups=[list(range(N_CORES))],
                    ins=[kvlocal[:]], outs=[kvgath[:]],
                )
                for j in range(2):
                    cj = ((pid + 6 + j) % N_CORES) * 128
                    nc.gpsimd.dma_start(
                        kT_rem[:, :, j * TC:(j + 1) * TC],
                        kvgath[bass.ds(cj, 128), 0:KCOLS].rearrange(
                            "p (a b) -> p a b", a=NH * 2))
                for j in range(2):
                    cj = ((pid + 6 + j) % N_CORES) * 128
                    nc.gpsimd.dma_start(
                        V_rem[:, NOT * j:NOT * (j + 1), :, :],
                        kvgath[bass.ds(cj, 128), KCOLS:KVCOLS].rearrange(
                            "p (a k c) -> p a k c", a=NOT, k=KV))

                # ------------- phase B1: Q projections (overlap gather) -------
                pend_q = None
                for n in range(NH):
                    wq_sb = wp.tile([128, NDT, H], BF16, tag="wh", name="wq")
                    nc.sync.dma_start(
                        wq_sb[:],
                        qw[:, n * H:(n + 1) * H].rearrange("(dt p) h -> p dt h", p=128))
                    psp = ps12.tile([128, 2, TC], F32, tag="pspair", name="pspQ")
                    for hh in range(2):
                        for dt in range(NDT):
                            nc.tensor.matmul(
                                psp[:, hh, :],
                                wq_sb[:, dt, hh * 128:(hh + 1) * 128],
                                xq_sb(dt),
                                start=(dt == 0), stop=(dt == NDT - 1))
                    if pend_q is not None:
                        pp, pn = pend_q
                        rope_norm_fold(pp, inv2q_sb, epsq1,
                                       qT_g[pn // 2][:, (pn % 2) * 2 + 0, :],
                                       qT_g[pn // 2][:, (pn % 2) * 2 + 1, :], "pe")
                    pend_q = (psp, n)
                pp, pn = pend_q
                rope_norm_fold(pp, inv2q_sb, epsq1,
                               qT_g[pn // 2][:, (pn % 2) * 2 + 0, :],
                               qT_g[pn // 2][:, (pn % 2) * 2 + 1, :], "pe")


            # ---------------- phase B2: attention ----------------
            with tc.tile_pool(name="p3", bufs=1) as p3, \
                 tc.tile_pool(name="aw", bufs=4) as aw, \
                 tc.tile_pool(name="ps3", bufs=2, space="PSUM") as ps3, \
                 tc.tile_pool(name="psenc", bufs=2, space="PSUM") as psenc:
                maskT_sb = p3.tile([128, NST, TC], F32)       # 24 KB/p
                mr = maskT.rearrange("j p t -> p j t")
                nc.sync.dma_start(maskT_sb[:, 0:NST // 2, :], mr[:, 0:NST // 2, :])
                nc.sync.dma_start(maskT_sb[:, NST // 2:, :], mr[:, NST // 2:, :])

                for g in range(KV):
                    heads = (2 * g, 2 * g + 1)
                    encs = [psenc.tile([128, 2, TC], F32, tag="enc",
                                       name=f"enc{a}") for a in range(2)]
                    denb = ps3.tile([1, 2 * TC], F32, tag="den", name="denb",
                                    bufs=1)
                    def pv_step(pTpair, st):
                        # enc.T[h, t] += V.T @ P.T ; den[t] += sum_s P.T
                        # only the in-window query-column range of this s-tile
                        lo, hi = max(0, 128 * (st - 8)), min(TC, 128 * (st + 1))
                        vsl = (V_own[:, st - 8, g, :] if st >= 8
                               else V_rem[:, st, g, :])
                        for hh in range(2):
                            for a in range(2):
                                nc.tensor.matmul(
                                    encs[a][:, hh, lo:hi],
                                    vsl[:, hh * 128:(hh + 1) * 128],
                                    pTpair[:, a, lo:hi],
                                    start=(st == 8), stop=(st == 7))
                        for a in range(2):
                            nc.tensor.matmul(
                                denb[:, a * TC + lo:a * TC + hi], ones_b[:],
                                pTpair[:, a, lo:hi],
                                start=(st == 8), stop=(st == 7))

                    pend_pv = None
                    for st in [8, 9, 10, 11, 0, 1, 2, 3, 4, 5, 6, 7]:
                        if st >= 8:
                            ksl = kT_own[:, :, (st - 8) * 128:(st - 7) * 128]
                        else:
                            ksl = kT_rem[:, :, st * 128:(st + 1) * 128]
                        lo, hi = max(0, 128 * (st - 8)), min(TC, 128 * (st + 1))
                        psLs = [ps3.tile([128, TC], F32, tag="psL",
                                         name=f"psL{a}") for a in range(2)]
                        for hh in range(2):
                            for a, n in enumerate(heads):
                                nc.tensor.matmul(
                                    psLs[a][:, lo:hi],
                                    ksl[:, g * 2 + hh, :],
                                    qT_g[g][:, a * 2 + hh, lo:hi],
                                    start=(hh == 0), stop=(hh == 1))
                        pTb = aw.tile([128, 2, TC], BF16, tag="pT", name="pTb",
                                      bufs=3)
                        t1s = [aw.tile([128, TC], F32, tag="t1", name=f"t1{a}",
                                       bufs=4) for a in range(2)]
                        for a in range(2):
                            nc.scalar.activation(t1s[a][:, lo:hi],
                                                 psLs[a][:, lo:hi], AF.Tanh,
                                                 scale=1.0 / SOFT_CAP)
                        for a in range(2):
                            nc.vector.tensor_tensor(t1s[a][:, lo:hi],
                                                    t1s[a][:, lo:hi],
                                                    maskT_sb[:, st, lo:hi],
                                                    OP.add)
                        for a in range(2):
                            nc.scalar.activation(pTb[:, a, lo:hi],
                                                 t1s[a][:, lo:hi], AF.Exp,
                                                 scale=SOFT_CAP)
                        if pend_pv is not None:
                            pv_step(*pend_pv)
                        pend_pv = (pTb, st)
                    pv_step(*pend_pv)
                    for a, n in enumerate(heads):
                        drow = aw.tile([1, TC], F32, tag="drow", name="drow", bufs=2)
                        nc.vector.reciprocal_approx_fast(drow[:], denb[:, a * TC:(a + 1) * TC])
                        rbden = aw.tile([128, TC], F32, tag="rbden", name="rbden", bufs=2)
                        nc.gpsimd.partition_broadcast(rbden[:], drow[:])
                        for hh in range(2):
                            nc.vector.tensor_tensor(
                                encT_sb[:, n * 2 + hh, :], encs[a][:, hh, :],
                                rbden[:], OP.mult)

            # ---------------- phase C: output projection ----------------
            with tc.tile_pool(name="outp", bufs=3) as outp, \
                 tc.tile_pool(name="ps4", bufs=4, space="PSUM") as ps4:
                for dc in range(4):
                    ow_sb = owp.tile([128, NH * 2, 512], BF16, tag="ow", name="ow_sb")
                    nc.sync.dma_start(
                        ow_sb[:],
                        ow[:, dc * 512:(dc + 1) * 512].rearrange(
                            "(nh p) d -> p nh d", p=128))
                    for tt in range(NTT):
                        psO = ps4.tile([128, 512], F32, tag="psO", name="psO")
                        for nh in range(NH * 2):
                            nc.tensor.matmul(
                                psO[:],
                                encT_sb[:, nh, tt * 128:(tt + 1) * 128],
                                ow_sb[:, nh, :],
                                start=(nh == 0), stop=(nh == NH * 2 - 1))
                        ob = outp.tile([128, 512], F32, tag="ob", name="ob")
                        nc.vector.tensor_copy(ob[:], psO[:])
                        nc.sync.dma_start(
                            out[tt * 128:(tt + 1) * 128, dc * 512:(dc + 1) * 512],
                            ob[:])

    nc.compile()
    return nc


_NC_CACHE = None


def _get_program():
    global _NC_CACHE
    if _NC_CACHE is None:
        _NC_CACHE = build_program()
    return _NC_CACHE


def prepare_inputs(x, q_w, kv_w, o_w, q_scale, k_scale, v_scale, segment_pos,
                   attn_mask):
    """Host-side prep: shard + transpose + fold scales + tables + masks."""
    x = np.asarray(x)
    q_w, kv_w, o_w = np.asarray(q_w), np.asarray(kv_w), np.asarray(o_w)
    q_scale, k_scale, v_scale = (np.asarray(q_scale), np.asarray(k_scale),
                                 np.asarray(v_scale))
    segment_pos = np.asarray(segment_pos)
    attn_mask = np.asarray(attn_mask)
    assert x.shape == (1, T, D)

    qs, ks, vs = 1.0 + q_scale, 1.0 + k_scale, 1.0 + v_scale
    qw_flat = (q_w * qs[None, None, :]).transpose(1, 0, 2).reshape(D, NH * H)
    kwk_flat = (kv_w[0] * ks[None, None, :]).transpose(1, 0, 2).reshape(D, KV * H)
    kwv_flat = (kv_w[1] * vs[None, None, :]).transpose(1, 0, 2).reshape(D, KV * H)
    ow_flat = o_w.reshape(NH * H, D)
    bf = ml_dtypes.bfloat16
    qw_b = np.ascontiguousarray(qw_flat, dtype=bf)
    kwk_b = np.ascontiguousarray(kwk_flat, dtype=bf)
    kwv_b = np.ascontiguousarray(kwv_flat, dtype=bf)
    ow_b = np.ascontiguousarray(ow_flat, dtype=bf)

    inv2q_arr = (qs ** -2.0).reshape(2, HH).T.astype(ml_dtypes.bfloat16)
    inv2k_arr = ((ks ** -2.0) / H).reshape(2, HH).T.astype(ml_dtypes.bfloat16)
    inv2v_arr = (np.tile(vs ** -2.0, KV) / H)[None, :].astype(np.float32)

    pos = segment_pos[0].astype(np.float64)
    freq = ROPE_BASE ** (2.0 * np.arange(HH) / H)
    xt_full = np.ascontiguousarray(x[0].T, dtype=bf)   # [D, T]
    am = attn_mask[0]                                  # [T, T] bool

    t_all = np.arange(T)
    in_maps = []
    for c in range(N_CORES):
        t_lo = c * TC
        xq_c = np.ascontiguousarray(xt_full[:, t_lo:t_lo + TC])

        ang = pos[t_lo:t_lo + TC][None, :] / freq[:, None]   # [HH, TC]
        cosq_c = np.cos(ang).astype(np.float32)
        sinq_c = np.sin(ang).astype(np.float32)

        s_idx = np.arange(t_lo - WINDOW, t_lo + TC)    # [SW]
        valid_s = s_idx >= 0
        sv = s_idx[valid_s]
        t_g = t_all[t_lo:t_lo + TC]
        m = np.zeros((SW, TC), dtype=bool)
        m[valid_s] = am[t_lo:t_lo + TC][:, sv].T
        dwin = t_g[None, :] - s_idx[:, None]
        m &= (dwin >= 0) & (dwin < WINDOW)
        maskT_c = np.where(m, 0.0, -4.0).astype(np.float32).reshape(NST, 128, TC)

        in_maps.append(dict(
            xq=xq_c, qw=qw_b, kwk=kwk_b, kwv=kwv_b, ow=ow_b,
            cosq=cosq_c, sinq=sinq_c, maskT=maskT_c,
            inv2q=inv2q_arr, inv2k=inv2k_arr, inv2v=inv2v_arr,
        ))
    return in_maps


def run(in_maps, trace=False, **kwargs):
    nc = _get_program()
    return run_bass_kernel_spmd(nc, in_maps, core_ids=list(range(N_CORES)),
                                trace=trace, **kwargs)


def kernel(**inputs) -> np.ndarray:
    in_maps = prepare_inputs(**inputs)
    res = run(in_maps)
    out = np.concatenate([res.results[c]["out"] for c in range(N_CORES)], axis=0)
    return out.reshape(1, T, D).astype(np.float32)


if __name__ == "__main__":
    nc = _get_program()
    print("built + compiled OK")
